# revision 1
# baseline (speedup 1.0000x reference)
"""DECConsLoss Trainium2 kernel: 8-core data-parallel over groups.

Reference computation (per group g of G=32, M=2048 tokens, C=512):
  ft_n, fc_n = l2norm(ft), l2norm(fc)          [M, C]
  grp[m]     = argmax_s grp_masks[s, m]        (S=16 slots)
  logits     = ft_n @ fc_n^T / 0.1             [M, M]
  lse[m]     = logsumexp(logits[m, :])
  semi[m]    = scale * (mean_{n: grp[n]==grp[m]} logits[m, n] - lse[m])
  pos[m]     = scale * (logits[m, m] - lse[m])
  loss       = mean(semi + pos) / 2,   scale = -(0.1/0.07)

Device-side decomposition (all compute on NeuronCores):
  - masked row-sums via a tiny side-GEMM: Q = onehot^T @ fc_n  [16, C],
    P = ft_n @ Q^T  [M, 16], masked_mean[m] = sum_s (onehot/cnt)[m,s]*P[m,s]
  - lse without max-subtraction (|logits| <= 10, fp32-safe)
  - diag via identity-masked fused multiply-reduce on the PSUM logits tile
  - rsqrt via exp(-0.5*ln(ssq)) on ScalarE
  - features cast to bf16 after normalization; GEMMs in bf16 (fp32 PSUM)
Each core handles 4 groups (= 8 consecutive (b,t) frames) and returns
per-partition-row partial sums [128, 1]; the host sums and scales.
"""

import sys
import numpy as np

for p in ("/opt/trn_rl_repo", "/opt/trn_rl_repo/concourse", "/opt/pypackages"):
    if p not in sys.path:
        sys.path.insert(0, p)

GF = 2          # group_frame
S = 16          # slots
N = 1024        # tokens per frame
C = 512         # feature dim
B, T = 8, 8
G = (B * T) // GF            # 32 groups total
M = GF * N                   # 2048 tokens per group
N_CORES = 8
GROUPS_PER_CORE = G // N_CORES   # 4
FRAMES_PER_CORE = GROUPS_PER_CORE * GF  # 8
TEMP = 0.1
BASE_TEMP = 0.07
INV_TEMP = 1.0 / TEMP        # 10.0
SCALE = -(TEMP / BASE_TEMP)

_CACHE = {}


def _build():
    import concourse.mybir as mybir
    from concourse import bacc
    from concourse import masks
    from concourse import bass_isa
    from concourse.tile import TileContext

    dt = mybir.dt
    Alu = mybir.AluOpType
    Act = mybir.ActivationFunctionType

    nc = bacc.Bacc()
    ft_d = nc.declare_dram_parameter("ft", [FRAMES_PER_CORE, N, C], dt.float32, isOutput=False)
    fc_d = nc.declare_dram_parameter("fc", [FRAMES_PER_CORE, N, C], dt.float32, isOutput=False)
    gm_d = nc.declare_dram_parameter("gm", [FRAMES_PER_CORE, S, N], dt.float32, isOutput=False)
    out_d = nc.declare_dram_parameter("out", [128, 2], dt.float32, isOutput=True)

    NT = M // 128       # 16 token tiles per group
    KC = C // 128       # 4 contraction chunks
    NB = M // 512       # 4 psum n-blocks per m-tile

    with TileContext(nc) as tc:
        with (
            tc.tile_pool(name="consts", bufs=1) as consts,
            tc.tile_pool(name="ftT_pool", bufs=2) as ftT_pool,
            tc.tile_pool(name="fcT_pool", bufs=2) as fcT_pool,
            tc.tile_pool(name="qt_pool", bufs=2) as qt_pool,
            tc.tile_pool(name="raw_pool", bufs=34) as raw_pool,
            tc.tile_pool(name="norm_pool", bufs=8) as norm_pool,
            tc.tile_pool(name="stat_pool", bufs=3) as stat_pool,
            tc.tile_pool(name="scr_pool", bufs=2) as scr_pool,
            tc.tile_pool(name="grp_pool", bufs=2) as grp_pool,
            tc.tile_pool(name="col_pool", bufs=3) as col_pool,
            tc.tile_pool(name="acc_pool", bufs=1) as acc_pool,
            tc.tile_pool(name="lg_psum", bufs=2, space="PSUM") as lg_psum,
            tc.tile_pool(name="tp_psum", bufs=2, space="PSUM") as tp_psum,
            tc.tile_pool(name="sm_psum", bufs=2, space="PSUM") as sm_psum,
        ):
            # ---- constants ----
            id_bf16 = consts.tile([128, 128], dt.bfloat16)
            id_f32 = consts.tile([128, 128], dt.float32)
            id16_f32 = consts.tile([S, S], dt.float32)
            id16_bf16 = consts.tile([S, S], dt.bfloat16)
            for t in (id_bf16, id_f32, id16_f32, id16_bf16):
                masks.make_identity(nc, t[:])

            acc = acc_pool.tile([128, 2], dt.float32)
            nc.vector.memset(acc[:], 0.0)

            for g in range(GROUPS_PER_CORE):
                # ============ group-mask phase: onehot + 1/cnt weights ============
                grp_sb = grp_pool.tile([S, M], dt.float32)
                nc.sync.dma_start(
                    out=grp_sb.rearrange("s (f n) -> s f n", f=GF),
                    in_=gm_d[2 * g : 2 * g + 2].rearrange("f s n -> s f n"),
                )
                grpT = grp_pool.tile([128, S * NT], dt.float32)   # token-major [128, 16] x 16
                for j in range(NT):
                    tpg = sm_psum.tile([128, S], dt.float32, tag="sm", name=f"tpg{g}_{j}")
                    nc.tensor.transpose(tpg[:], grp_sb[:, j * 128 : (j + 1) * 128], id16_f32[:])
                    nc.vector.tensor_copy(grpT[:, j * S : (j + 1) * S], tpg[:])
                rowmax = stat_pool.tile([128, NT], dt.float32)
                oh_f32 = grp_pool.tile([128, S * NT], dt.float32)
                oh_bf16 = grp_pool.tile([128, S * NT], dt.bfloat16)
                oh_w = grp_pool.tile([128, S * NT], dt.float32)
                ohsum = stat_pool.tile([128, S], dt.float32)
                cntb = stat_pool.tile([128, S], dt.float32)
                for j in range(NT):
                    sl = slice(j * S, (j + 1) * S)
                    nc.vector.tensor_reduce(
                        out=rowmax[:, j : j + 1], in_=grpT[:, sl],
                        axis=mybir.AxisListType.X, op=Alu.max,
                    )
                    nc.vector.tensor_scalar(
                        out=oh_f32[:, sl], in0=grpT[:, sl],
                        scalar1=rowmax[:, j : j + 1], scalar2=None, op0=Alu.is_equal,
                    )
                    nc.vector.tensor_copy(oh_bf16[:, sl], oh_f32[:, sl])
                    if j == 0:
                        nc.vector.tensor_copy(ohsum[:], oh_f32[:, sl])
                    else:
                        nc.vector.tensor_tensor(out=ohsum[:], in0=ohsum[:], in1=oh_f32[:, sl], op=Alu.add)
                nc.gpsimd.partition_all_reduce(
                    out_ap=cntb[:], in_ap=ohsum[:], channels=128, reduce_op=bass_isa.ReduceOp.add,
                )
                nc.vector.tensor_scalar(out=cntb[:], in0=cntb[:], scalar1=1.0, scalar2=None, op0=Alu.max)
                nc.vector.reciprocal(out=cntb[:], in_=cntb[:])
                for j in range(NT):
                    sl = slice(j * S, (j + 1) * S)
                    nc.vector.tensor_tensor(out=oh_w[:, sl], in0=oh_f32[:, sl], in1=cntb[:], op=Alu.mult)

                # ============ load + sum-of-squares for BOTH tensors ============
                # (both ssq's first so the Ln/Exp rnorm ops cluster by function,
                #  minimizing ACT table-set reloads)
                fcT = fcT_pool.tile([128, KC * M], dt.bfloat16)
                qq = sm_psum.tile([S, C], dt.float32, tag="sm", name=f"qq{g}")
                fc_raws = []
                ssq_fc = stat_pool.tile([128, NT], dt.float32)
                for j in range(NT):
                    fc_raw = raw_pool.tile([128, C], dt.float32, tag="raw", name=f"fcraw{g}_{j}")
                    fc_raws.append(fc_raw)
                    nc.sync.dma_start(out=fc_raw[:], in_=fc_d[2 * g + j // 8, (j % 8) * 128 : (j % 8 + 1) * 128, :])
                    sq_scr = scr_pool.tile([128, C], dt.float32, tag="sq")
                    nc.scalar.activation(sq_scr[:], fc_raw[:], Act.Square, accum_out=ssq_fc[:, j : j + 1])
                ft_raws = []
                ssq_ft = stat_pool.tile([128, NT], dt.float32)
                for j in range(NT):
                    ft_raw = raw_pool.tile([128, C], dt.float32, tag="raw", name=f"ftraw{g}_{j}")
                    ft_raws.append(ft_raw)
                    nc.sync.dma_start(out=ft_raw[:], in_=ft_d[2 * g + j // 8, (j % 8) * 128 : (j % 8 + 1) * 128, :])
                    sq_scr = scr_pool.tile([128, C], dt.float32, tag="sq")
                    nc.scalar.activation(sq_scr[:], ft_raw[:], Act.Square, accum_out=ssq_ft[:, j : j + 1])
                rn_fc = stat_pool.tile([128, NT], dt.float32)
                rn_ft = stat_pool.tile([128, NT], dt.float32)
                nc.vector.tensor_scalar(out=rn_fc[:], in0=ssq_fc[:], scalar1=1e-24, scalar2=None, op0=Alu.max)
                nc.vector.tensor_scalar(out=rn_ft[:], in0=ssq_ft[:], scalar1=1e-24, scalar2=None, op0=Alu.max)
                nc.scalar.activation(rn_fc[:], rn_fc[:], Act.Ln)
                nc.scalar.activation(rn_ft[:], rn_ft[:], Act.Ln)
                nc.scalar.activation(rn_fc[:], rn_fc[:], Act.Exp, scale=-0.5)
                nc.scalar.activation(rn_ft[:], rn_ft[:], Act.Exp, scale=-0.5)

                # ============ fc: normalize + Q-GEMM + transpose ============
                for j in range(NT):
                    fcn = norm_pool.tile([128, C], dt.bfloat16, tag="normed", name=f"fcn{g}_{j}")
                    nc.vector.tensor_scalar(
                        out=fcn[:], in0=fc_raws[j][:], scalar1=rn_fc[:, j : j + 1], scalar2=None, op0=Alu.mult,
                    )
                    nc.tensor.matmul(
                        qq[:], oh_bf16[:, j * S : (j + 1) * S], fcn[:],
                        start=(j == 0), stop=(j == NT - 1),
                    )
                    tp = tp_psum.tile([128, C], dt.float32, tag="tp")
                    for k in range(KC):
                        nc.tensor.matmul(
                            tp[:, k * 128 : (k + 1) * 128], fcn[:, k * 128 : (k + 1) * 128], id_bf16[:],
                            start=True, stop=True,
                        )
                    nc.vector.tensor_copy(
                        fcT.rearrange("p (k m) -> p k m", k=KC)[:, :, j * 128 : (j + 1) * 128],
                        tp.rearrange("p (k m) -> p k m", k=KC),
                    )

                # ============ Q finalize: bf16 + transpose to [C, S] chunks ============
                q_sb = grp_pool.tile([S, C], dt.bfloat16)
                nc.vector.tensor_copy(q_sb[:], qq[:])
                qt = qt_pool.tile([128, KC * S], dt.bfloat16)
                for k in range(KC):
                    tp2 = sm_psum.tile([128, S], dt.float32, tag="sm", name=f"tp2{g}_{k}")
                    nc.tensor.matmul(tp2[:], q_sb[:, k * 128 : (k + 1) * 128], id16_bf16[:], start=True, stop=True)
                    nc.vector.tensor_copy(qt[:, k * S : (k + 1) * S], tp2[:])

                # ============ ft: normalize + transpose ============
                ftT = ftT_pool.tile([128, KC * M], dt.bfloat16)
                for j in range(NT):
                    ftn = norm_pool.tile([128, C], dt.bfloat16, tag="normed", name=f"ftn{g}_{j}")
                    nc.vector.tensor_scalar(
                        out=ftn[:], in0=ft_raws[j][:], scalar1=rn_ft[:, j : j + 1], scalar2=None, op0=Alu.mult,
                    )
                    tp = tp_psum.tile([128, C], dt.float32, tag="tp")
                    for k in range(KC):
                        nc.tensor.matmul(
                            tp[:, k * 128 : (k + 1) * 128], ftn[:, k * 128 : (k + 1) * 128], id_bf16[:],
                            start=True, stop=True,
                        )
                    nc.vector.tensor_copy(
                        ftT.rearrange("p (k m) -> p k m", k=KC)[:, :, j * 128 : (j + 1) * 128],
                        tp.rearrange("p (k m) -> p k m", k=KC),
                    )

                # ============ main phase: logits GEMM + LSE + masked means ============
                stot_all = stat_pool.tile([128, NT], dt.float32)
                for i in range(NT):
                    lhs = [ftT[:, k * M + i * 128 : k * M + (i + 1) * 128] for k in range(KC)]
                    lgs = [
                        lg_psum.tile([128, 1024], dt.float32, tag="lg", name=f"lg{g}_{i}_{h}")
                        for h in range(2)
                    ]
                    for nb in range(NB):
                        lg = lgs[nb // 2][:, (nb % 2) * 512 : (nb % 2 + 1) * 512]
                        for k in range(KC):
                            nc.tensor.matmul(
                                lg, lhs[k], fcT[:, k * M + nb * 512 : k * M + (nb + 1) * 512],
                                start=(k == 0), stop=(k == KC - 1),
                            )
                    pp = sm_psum.tile([128, S], dt.float32, tag="sm", name=f"pp{g}_{i}")
                    for k in range(KC):
                        nc.tensor.matmul(
                            pp[:], lhs[k], qt[:, k * S : (k + 1) * S],
                            start=(k == 0), stop=(k == KC - 1),
                        )
                    # diagonal (cosine units) from the block that contains it
                    diagc = col_pool.tile([128, 1], dt.float32, tag="diagc")
                    ttr_scr = scr_pool.tile([128, 128], dt.float32, tag="ttr")
                    doff = ((i // 4) % 2) * 512 + (i % 4) * 128
                    nc.vector.tensor_tensor(
                        out=ttr_scr[:], in0=lgs[i // 8][:, doff : doff + 128],
                        in1=id_f32[:], op=Alu.mult,
                    )
                    nc.vector.tensor_reduce(
                        out=diagc[:], in_=ttr_scr[:], axis=mybir.AxisListType.X, op=Alu.add,
                    )
                    # exp (scale=1/T) + row-sum accumulation
                    scols = col_pool.tile([128, 2], dt.float32, tag="scols")
                    for h in range(2):
                        exp_scr = scr_pool.tile([128, 1024], dt.bfloat16, tag="exp")
                        nc.scalar.activation(
                            exp_scr[:], lgs[h][:], Act.Exp, scale=INV_TEMP,
                            accum_out=scols[:, h : h + 1],
                        )
                    nc.vector.tensor_reduce(
                        out=stot_all[:, i : i + 1], in_=scols[:], axis=mybir.AxisListType.X, op=Alu.add,
                    )
                    # masked mean (cosine units): sum_s oh_w * P
                    mavg = col_pool.tile([128, 1], dt.float32, tag="mavg")
                    pttr_scr = scr_pool.tile([128, S], dt.float32, tag="pttr")
                    nc.vector.tensor_tensor(
                        out=pttr_scr[:], in0=pp[:], in1=oh_w[:, i * S : (i + 1) * S], op=Alu.mult,
                    )
                    nc.vector.tensor_reduce(
                        out=mavg[:], in_=pttr_scr[:], axis=mybir.AxisListType.X, op=Alu.add,
                    )
                    # acc col0 += mavg + diag (cosine units); lse batched after loop
                    t1 = col_pool.tile([128, 1], dt.float32, tag="t1")
                    nc.vector.tensor_tensor(out=t1[:], in0=mavg[:], in1=diagc[:], op=Alu.add)
                    nc.vector.tensor_tensor(out=acc[:, 0:1], in0=acc[:, 0:1], in1=t1[:], op=Alu.add)

                lse_all = stat_pool.tile([128, NT], dt.float32)
                nc.scalar.activation(lse_all[:], stot_all[:], Act.Ln)
                lsum = col_pool.tile([128, 1], dt.float32, tag="lsum")
                nc.vector.tensor_reduce(out=lsum[:], in_=lse_all[:], axis=mybir.AxisListType.X, op=Alu.add)
                nc.vector.tensor_tensor(out=acc[:, 1:2], in0=acc[:, 1:2], in1=lsum[:], op=Alu.add)

            nc.sync.dma_start(out=out_d[:, :], in_=acc[:])

    nc.compile()
    return nc


def kernel(feat_trainable: np.ndarray, feat_criterion: np.ndarray, grp_masks: np.ndarray) -> np.ndarray:
    from concourse.bass_utils import run_bass_kernel_spmd

    if "nc" not in _CACHE:
        _CACHE["nc"] = _build()
    nc = _CACHE["nc"]

    ft = np.ascontiguousarray(np.asarray(feat_trainable, dtype=np.float32).reshape(B * T, N, C))
    fc = np.ascontiguousarray(np.asarray(feat_criterion, dtype=np.float32).reshape(B * T, N, C))
    gm = np.ascontiguousarray(np.asarray(grp_masks, dtype=np.float32).reshape(B * T, S, N))

    in_maps = []
    for c in range(N_CORES):
        fr = slice(c * FRAMES_PER_CORE, (c + 1) * FRAMES_PER_CORE)
        in_maps.append({
            "ft": np.ascontiguousarray(ft[fr]),
            "fc": np.ascontiguousarray(fc[fr]),
            "gm": np.ascontiguousarray(gm[fr]),
        })

    import time
    last_err = None
    for attempt in range(4):
        try:
            res = run_bass_kernel_spmd(nc, in_maps, list(range(N_CORES)))
            break
        except Exception as e:  # wedged-device recovery: wait and retry
            last_err = e
            time.sleep(20 + 25 * attempt)
    else:
        raise last_err
    total = np.float64(0.0)
    for c in range(N_CORES):
        o = np.asarray(res.results[c]["out"], dtype=np.float64)
        total += INV_TEMP * o[:, 0].sum() - 2.0 * o[:, 1].sum()
    loss = SCALE * total / (G * M) / 2.0
    return np.asarray(loss, dtype=np.float32)


if __name__ == "__main__":
    # build-only smoke test
    nc = _build()
    print("build OK")



# revision 2
# speedup vs baseline: 1.5009x; 1.5009x over previous
"""DECConsLoss Trainium2 kernel v3: 8-core data-parallel over groups.

Reference (per group g of G=32, M=2048 tokens, C=512):
  ft_n, fc_n = l2norm(ft), l2norm(fc);  grp[m] = argmax_s masks
  logits = ft_n @ fc_n^T / 0.1;  lse[m] = logsumexp(logits[m,:])
  semi[m] = scale*(mean_{n: grp[n]==grp[m]} logits[m,n] - lse)
  pos[m]  = scale*(logits[m,m] - lse);  loss = mean(semi+pos)/2

v3 pipeline (all primitives hardware-validated):
  - SWDGE casting DMA loads fp32 DRAM -> bf16 SBUF token-major [128,(j,c)]
  - DMA XBAR transposes bf16 SBUF->SBUF -> [128,(j,k,m)] ("alt" layout)
  - ssq via plain bf16 Gram matmuls; diag extracted by tensor_tensor
    against a 4-wide identity + segmented tensor_reduce
  - 16*rsqrt(ssq) via Newton iteration on gpsimd (no ACT table thrash)
  - both tensors scaled to fp8 with broadcast matrices A (j-major out,
    gating per chunk) / B (k-major out, gating per n-half); exp scale is
    then the constant 10/256
  - main loop runs n-half-outer so the first 16 exps only need fc's
    first-half chain; ft's full chain is shorter
  - per-i diag from fp8 DR cross-Grams; masked means via Q/P side-GEMMs
  - exp in-place on PSUM with accum_out row-sums; ln(sums) on the host
Host: loss = SCALE*((10/256)*sum(acc0) - 2*sum(ln(stot))) / (G*M) / 2.
"""

import sys
import numpy as np

for p in ("/opt/trn_rl_repo", "/opt/trn_rl_repo/concourse", "/opt/pypackages"):
    if p not in sys.path:
        sys.path.insert(0, p)

GF = 2
S = 16
N = 1024
C = 512
B, T = 8, 8
G = (B * T) // GF            # 32 groups total
M = GF * N                   # 2048 tokens per group
N_CORES = 8
GROUPS_PER_CORE = G // N_CORES   # 4
FRAMES_PER_CORE = GROUPS_PER_CORE * GF  # 8
TEMP = 0.1
BASE_TEMP = 0.07
INV_TEMP = 1.0 / TEMP
SCALE = -(TEMP / BASE_TEMP)
FS = 16.0                    # fp8 pre-scale on both normalized tensors
POST = INV_TEMP / (FS * FS)  # 10/256: logits = POST * H

NT = M // 128                # 16 token tiles per group
KC = C // 128                # 4 contraction chunks

_CACHE = {}


def _build():
    import math
    import concourse.mybir as mybir
    from concourse import bacc
    from concourse import masks
    from concourse import bass_isa
    from concourse.tile import TileContext

    dt = mybir.dt
    Alu = mybir.AluOpType
    Act = mybir.ActivationFunctionType
    DR = mybir.MatmulPerfMode.DoubleRow

    nc = bacc.Bacc()
    ft_d = nc.declare_dram_parameter("ft", [FRAMES_PER_CORE, N, C], dt.float32, isOutput=False)
    fc_d = nc.declare_dram_parameter("fc", [FRAMES_PER_CORE, N, C], dt.float32, isOutput=False)
    gm_d = nc.declare_dram_parameter("gm", [FRAMES_PER_CORE, S, N], dt.float32, isOutput=False)
    out_d = nc.declare_dram_parameter("out", [128, 16 + 32 * GROUPS_PER_CORE], dt.float32, isOutput=True)

    f8 = dt.float8e4
    bf = dt.bfloat16

    with TileContext(nc) as tc:
        with (
            tc.tile_pool(name="consts", bufs=1) as consts,
            tc.tile_pool(name="tok_pool", bufs=2) as tok_pool,
            tc.tile_pool(name="tT_pool", bufs=2) as tT_pool,
            tc.tile_pool(name="n8_pool", bufs=2) as n8_pool,
            tc.tile_pool(name="ab_pool", bufs=1) as ab_pool,
            tc.tile_pool(name="grp_pool", bufs=2) as grp_pool,
            tc.tile_pool(name="stat_pool", bufs=2) as stat_pool,
            tc.tile_pool(name="scr_pool", bufs=2) as scr_pool,
            tc.tile_pool(name="q_pool", bufs=2) as q_pool,
            tc.tile_pool(name="acc_pool", bufs=1) as acc_pool,
            tc.tile_pool(name="sm_psum", bufs=2, space="PSUM") as sm_psum,
            tc.tile_pool(name="lg_psum", bufs=3, space="PSUM") as lg_psum,
        ):
            # ---- constants ----
            id128_bf = consts.tile([128, 128], bf)
            id128_f32 = consts.tile([128, 128], dt.float32)
            id16_bf = consts.tile([S, S], bf)
            id16_f32 = consts.tile([S, S], dt.float32)
            for t in (id128_bf, id128_f32, id16_bf, id16_f32):
                masks.make_identity(nc, t[:])
            id4_bf = consts.tile([128, 512], bf)
            for q in range(4):
                nc.gpsimd.tensor_copy(id4_bf[:, q * 128 : (q + 1) * 128], id128_bf[:])
            # E8[:, jj*128:(jj+1)*128] = row-jj selector over 8 partitions
            E8 = consts.tile([8, 8 * 128], bf)
            nc.gpsimd.memset(E8[:], 0.0)
            nc.gpsimd.affine_select(
                out=E8[:], in_=E8[:],
                compare_op=mybir.AluOpType.not_equal, fill=1.0, base=0,
                pattern=[[-1, 8], [0, 128]], channel_multiplier=1,
            )

            acc = acc_pool.tile([128, 16 + 32 * GROUPS_PER_CORE], dt.float32)
            nc.vector.memset(acc[:, 0:16], 0.0)

            w0 = FS / math.sqrt(512.0)

            def newton16(ssq, dst, cols):
                """dst[:, cols] = FS*rsqrt(max(ssq,eps)) via 3 Newton steps."""
                sq, dv = ssq[:, cols], dst[:, cols]
                wscr = stat_pool.tile([128, 8], dt.float32, tag="wscr")
                nc.gpsimd.tensor_scalar(
                    out=sq, in0=sq, scalar1=1e-12, scalar2=1.0 / (FS * FS),
                    op0=Alu.max, op1=Alu.mult,
                )
                nc.gpsimd.tensor_scalar(
                    out=dv, in0=sq, scalar1=-0.5 * w0 ** 3, scalar2=1.5 * w0,
                    op0=Alu.mult, op1=Alu.add,
                )
                for _ in range(2):
                    nc.gpsimd.tensor_tensor(out=wscr[:], in0=dv, in1=dv, op=Alu.mult)
                    nc.gpsimd.tensor_tensor(out=wscr[:], in0=wscr[:], in1=sq, op=Alu.mult)
                    nc.gpsimd.tensor_scalar(
                        out=wscr[:], in0=wscr[:], scalar1=-0.5, scalar2=1.5,
                        op0=Alu.mult, op1=Alu.add,
                    )
                    nc.gpsimd.tensor_tensor(out=dv, in0=dv, in1=wscr[:], op=Alu.mult)

            def bcast_half(src, mat, h, g, nm):
                """mat[:, h*1024:(h+1)*1024] <- broadcast rows of src[:, h*8:(h+1)*8]."""
                tps = sm_psum.tile([8, 128], dt.float32, tag="sm", name=f"{nm}tp{g}_{h}")
                nc.tensor.transpose(tps[:], src[:, h * 8 : (h + 1) * 8], id128_f32[:])
                row_sb = q_pool.tile([8, 128], bf, tag=f"{nm}row")
                nc.vector.tensor_copy(row_sb[:], tps[:])
                for half_q in range(2):
                    bps = sm_psum.tile([128, 512], dt.float32, tag="sm", name=f"{nm}ps{g}_{h}_{half_q}")
                    for jj in range(4):
                        j = half_q * 4 + jj
                        nc.tensor.matmul(
                            bps[:, jj * 128 : (jj + 1) * 128],
                            E8[:, j * 128 : (j + 1) * 128], row_sb[:],
                            start=True, stop=True,
                        )
                    nc.vector.tensor_copy(
                        mat[:, h * 1024 + half_q * 512 : h * 1024 + (half_q + 1) * 512], bps[:]
                    )

            def gram_bank(T_bf, ssq, i0, g, nm, src3=None):
                gm = sm_psum.tile([128, 512], dt.float32, tag="sm", name=f"{nm}{g}_{i0}")
                for di in range(4):
                    i = i0 + di
                    if src3 is None:
                        for k in range(KC):
                            sl = slice(i * 512 + k * 128, i * 512 + (k + 1) * 128)
                            nc.tensor.matmul(
                                gm[:, di * 128 : (di + 1) * 128],
                                T_bf[:, sl], T_bf[:, sl],
                                start=(k == 0), stop=(k == KC - 1),
                            )
                    else:
                        lhs3, rhs3 = src3
                        for kp in (0, 2):
                            nc.tensor.matmul(
                                gm[:, di * 128 : (di + 1) * 128],
                                lhs3[:, kp : kp + 2, i * 128 : (i + 1) * 128],
                                rhs3[:, kp : kp + 2, i * 128 : (i + 1) * 128],
                                start=(kp == 0), stop=(kp == 2), perf_mode=DR,
                            )
                scr = scr_pool.tile([128, 512], bf, tag="scr")
                nc.vector.tensor_tensor(out=scr[:], in0=gm[:], in1=id4_bf[:], op=Alu.mult)
                nc.vector.tensor_reduce(
                    out=ssq[:, i0 : i0 + 4],
                    in_=scr.rearrange("p (q m) -> p q m", q=4),
                    axis=mybir.AxisListType.X, op=Alu.add,
                )

            for g in range(GROUPS_PER_CORE):
                # ============ phase B: fc chain ============
                fc_bf = tok_pool.tile([128, NT * C], bf, tag="fcbf", name=f"fcbf{g}")
                fcT = tT_pool.tile([128, NT * C], bf, tag="fcT", name=f"fcT{g}")
                ssq_fc = stat_pool.tile([128, NT], dt.float32, tag="ssq_fc")
                bprime = stat_pool.tile([128, NT], dt.float32, tag="bprime")
                B_sb = ab_pool.tile([128, M], bf, tag="Bmat")
                fcn = n8_pool.tile([128, KC * M], f8, tag="fcn", name=f"fcn{g}")
                nc.gpsimd.dma_start(
                    out=fc_bf.rearrange("p (j c) -> p j c", j=NT),
                    in_=fc_d[2 * g : 2 * g + 2].rearrange("f (jj p) c -> p (f jj) c", p=128),
                )
                nc.sync.dma_start_transpose(
                    out=fcT.rearrange("p (q m) -> p q m", m=128), in_=fc_bf[:]
                )
                for i0 in range(0, NT, 4):
                    gram_bank(fcT, ssq_fc, i0, g, "cg")
                for h in range(2):
                    newton16(ssq_fc, bprime, slice(h * 8, (h + 1) * 8))
                    bcast_half(bprime, B_sb, h, g, "B")
                # fcn k-major: out col = k*2048 + n
                for h in range(2):
                    for k in range(KC):
                        eng = nc.vector if k % 2 == 0 else nc.gpsimd
                        eng.tensor_tensor(
                            out=fcn[:, k * M + h * 1024 : k * M + (h + 1) * 1024]
                                .rearrange("p (j m) -> p j m", m=128),
                            in0=fcT.rearrange("p (j km) -> p j km", j=NT)
                                [:, h * 8 : (h + 1) * 8, k * 128 : (k + 1) * 128],
                            in1=B_sb[:, h * 1024 : (h + 1) * 1024]
                                .rearrange("p (j m) -> p j m", m=128),
                            op=Alu.mult,
                        )
                fcn3 = fcn.rearrange("p (k n) -> p k n", k=KC)

                # ============ phase A: ft full chain (gates lhsT side) ============
                ft_bf = tok_pool.tile([128, NT * C], bf, tag="ftbf", name=f"ftbf{g}")
                nc.gpsimd.dma_start(
                    out=ft_bf.rearrange("p (j c) -> p j c", j=NT),
                    in_=ft_d[2 * g : 2 * g + 2].rearrange("f (jj p) c -> p (f jj) c", p=128),
                )
                ftT = tT_pool.tile([128, NT * C], bf, tag="ftT", name=f"ftT{g}")
                nc.sync.dma_start_transpose(
                    out=ftT.rearrange("p (q m) -> p q m", m=128), in_=ft_bf[:]
                )
                ssq_ft = stat_pool.tile([128, NT], dt.float32, tag="ssq_ft")
                for i0 in range(0, NT, 4):
                    gram_bank(ftT, ssq_ft, i0, g, "fg")
                ahat = stat_pool.tile([128, NT], dt.float32, tag="ahat")
                for h in range(2):
                    newton16(ssq_ft, ahat, slice(h * 8, (h + 1) * 8))
                A_sb = ab_pool.tile([128, M], bf, tag="Amat")
                for h in range(2):
                    bcast_half(ahat, A_sb, h, g, "A")
                # ftn in alt (j-major) layout: col = j*512 + k*128 + m
                ftn = n8_pool.tile([128, KC * M], f8, tag="ftn", name=f"ftn{g}")
                for c in range(4):
                    for k in range(KC):
                        eng = nc.vector if k % 2 == 0 else nc.gpsimd
                        eng.tensor_tensor(
                            out=ftn.rearrange("p (j km) -> p j km", j=NT)
                                [:, c * 4 : (c + 1) * 4, k * 128 : (k + 1) * 128],
                            in0=ftT.rearrange("p (j km) -> p j km", j=NT)
                                [:, c * 4 : (c + 1) * 4, k * 128 : (k + 1) * 128],
                            in1=A_sb[:, c * 512 : (c + 1) * 512]
                                .rearrange("p (j m) -> p j m", m=128),
                            op=Alu.mult,
                        )

                # ============ main: logits GEMM (DR) + exp, n-half outer ============
                stot2 = stat_pool.tile([128, 2 * NT], dt.float32, tag="stot2")
                for i in range(NT):
                    for h in range(2):
                        lg = lg_psum.tile([128, 1024], dt.float32, tag="lg", name=f"lg{g}_{i}_{h}")
                        for nb in range(2):
                            blk = lg[:, nb * 512 : (nb + 1) * 512]
                            ncol = (2 * h + nb) * 512
                            for kp in (0, 2):
                                nc.tensor.matmul(
                                    blk,
                                    ftn[:, i * 512 + kp * 128 : i * 512 + (kp + 2) * 128]
                                        .rearrange("p (k m) -> p k m", k=2),
                                    fcn3[:, kp : kp + 2, ncol : ncol + 512],
                                    start=(kp == 0), stop=(kp == 2), perf_mode=DR,
                                )
                        nc.scalar.activation(
                            lg[:], lg[:], Act.Exp, scale=POST,
                            accum_out=stot2[:, 2 * i + h : 2 * i + h + 1],
                        )

                # ============ phase D: diag cross-Grams, Q, pp/mavg ============
                # grp-mask work is independent; emitted here so its engine use
                # overlaps the ft/fc chains above.
                grp_sb = grp_pool.tile([S, M], dt.float32, tag="grp_sb")
                nc.sync.dma_start(
                    out=grp_sb.rearrange("s (f n) -> s f n", f=GF),
                    in_=gm_d[2 * g : 2 * g + 2].rearrange("f s n -> s f n"),
                )
                tpg = sm_psum.tile([128, NT * S], dt.float32, tag="sm", name=f"tpg{g}")
                for j in range(NT):
                    nc.tensor.transpose(
                        tpg[:, j * S : (j + 1) * S], grp_sb[:, j * 128 : (j + 1) * 128], id16_f32[:]
                    )
                grpT = grp_pool.tile([128, NT * S], dt.float32, tag="grpT")
                nc.vector.tensor_copy(grpT[:], tpg[:])
                rowmax = stat_pool.tile([128, NT], dt.float32, tag="rowmax")
                nc.vector.tensor_reduce(
                    out=rowmax[:],
                    in_=grpT.rearrange("p (j s) -> p j s", j=NT),
                    axis=mybir.AxisListType.X, op=Alu.max,
                )
                oh_f32 = grp_pool.tile([128, NT * S], dt.float32, tag="oh_f32")
                oh_w = grp_pool.tile([128, NT * S], dt.float32, tag="oh_w")
                ohsum = stat_pool.tile([128, S], dt.float32, tag="ohsum")
                cntb = stat_pool.tile([128, S], dt.float32, tag="cntb")
                for j in range(NT):
                    sl = slice(j * S, (j + 1) * S)
                    nc.gpsimd.tensor_scalar(
                        out=oh_f32[:, sl], in0=grpT[:, sl],
                        scalar1=rowmax[:, j : j + 1], scalar2=None, op0=Alu.is_equal,
                    )
                    if j == 0:
                        nc.gpsimd.tensor_copy(ohsum[:], oh_f32[:, sl])
                    else:
                        nc.gpsimd.tensor_tensor(out=ohsum[:], in0=ohsum[:], in1=oh_f32[:, sl], op=Alu.add)
                nc.gpsimd.partition_all_reduce(
                    out_ap=cntb[:], in_ap=ohsum[:], channels=128, reduce_op=bass_isa.ReduceOp.add,
                )
                nc.gpsimd.tensor_scalar(out=cntb[:], in0=cntb[:], scalar1=1.0, scalar2=None, op0=Alu.max)
                nc.vector.reciprocal(out=cntb[:], in_=cntb[:])
                for j in range(NT):
                    sl = slice(j * S, (j + 1) * S)
                    nc.gpsimd.tensor_tensor(out=oh_w[:, sl], in0=oh_f32[:, sl], in1=cntb[:], op=Alu.mult)

                # cross-Gram diag (ftn alt-layout lhsT, fcn k-major rhs)
                diagF = stat_pool.tile([128, NT], dt.float32, tag="diagF")
                for i0 in range(0, NT, 4):
                    gmx = sm_psum.tile([128, 512], dt.float32, tag="sm", name=f"xg{g}_{i0}")
                    for di in range(4):
                        i = i0 + di
                        for kp in (0, 2):
                            nc.tensor.matmul(
                                gmx[:, di * 128 : (di + 1) * 128],
                                ftn[:, i * 512 + kp * 128 : i * 512 + (kp + 2) * 128]
                                    .rearrange("p (k m) -> p k m", k=2),
                                fcn3[:, kp : kp + 2, i * 128 : (i + 1) * 128],
                                start=(kp == 0), stop=(kp == 2), perf_mode=DR,
                            )
                    scr = scr_pool.tile([128, 512], bf, tag="scr")
                    nc.vector.tensor_tensor(out=scr[:], in0=gmx[:], in1=id4_bf[:], op=Alu.mult)
                    nc.vector.tensor_reduce(
                        out=diagF[:, i0 : i0 + 4],
                        in_=scr.rearrange("p (q m) -> p q m", q=4),
                        axis=mybir.AxisListType.X, op=Alu.add,
                    )

                oh_b = grp_pool.tile([128, NT * S], bf, tag="oh_b")
                for j in range(NT):
                    sl = slice(j * S, (j + 1) * S)
                    nc.gpsimd.tensor_scalar(
                        out=oh_b[:, sl], in0=oh_f32[:, sl],
                        scalar1=bprime[:, j : j + 1], scalar2=None, op0=Alu.mult,
                    )
                qq = sm_psum.tile([S, C], dt.float32, tag="sm", name=f"qq{g}")
                for j in range(NT):
                    nc.tensor.matmul(
                        qq[:], oh_b[:, j * S : (j + 1) * S], fc_bf[:, j * C : (j + 1) * C],
                        start=(j == 0), stop=(j == NT - 1),
                    )
                q_sb = q_pool.tile([S, C], bf, tag="q_sb")
                nc.vector.tensor_copy(q_sb[:], qq[:])
                qtp = sm_psum.tile([128, KC * S], bf, tag="sm", name=f"qtp{g}")
                for k in range(KC):
                    nc.tensor.transpose(
                        qtp[:, k * S : (k + 1) * S], q_sb[:, k * 128 : (k + 1) * 128], id16_bf[:]
                    )
                qt_sb = q_pool.tile([128, KC * S], bf, tag="qt_sb")
                nc.vector.tensor_copy(qt_sb[:], qtp[:])

                mavg = stat_pool.tile([128, NT], dt.float32, tag="mavg")
                for i0 in range(0, NT, 4):
                    ppt = sm_psum.tile([128, 4 * S], dt.float32, tag="sm", name=f"pp{g}_{i0}")
                    for di in range(4):
                        i = i0 + di
                        for k in range(KC):
                            nc.tensor.matmul(
                                ppt[:, di * S : (di + 1) * S],
                                ftn[:, i * 512 + k * 128 : i * 512 + (k + 1) * 128],
                                qt_sb[:, k * S : (k + 1) * S],
                                start=(k == 0), stop=(k == KC - 1),
                            )
                    pscr = scr_pool.tile([128, 4 * S], dt.float32, tag="pscr")
                    nc.vector.tensor_tensor(
                        out=pscr[:], in0=ppt[:], in1=oh_w[:, i0 * S : (i0 + 4) * S], op=Alu.mult
                    )
                    nc.vector.tensor_reduce(
                        out=mavg[:, i0 : i0 + 4],
                        in_=pscr.rearrange("p (q s) -> p q s", q=4),
                        axis=mybir.AxisListType.X, op=Alu.add,
                    )

                # ============ finals (lse ln'd host-side from raw stot2) ============
                tsum = stat_pool.tile([128, NT], dt.float32, tag="tsum")
                nc.gpsimd.tensor_tensor(out=tsum[:], in0=diagF[:], in1=mavg[:], op=Alu.add)
                nc.gpsimd.tensor_tensor(out=acc[:, 0:NT], in0=acc[:, 0:NT], in1=tsum[:], op=Alu.add)
                nc.gpsimd.tensor_copy(acc[:, 16 + 32 * g : 16 + 32 * (g + 1)], stot2[:])

            nc.sync.dma_start(out=out_d[:, :], in_=acc[:])

    nc.compile()
    return nc


def kernel(feat_trainable: np.ndarray, feat_criterion: np.ndarray, grp_masks: np.ndarray) -> np.ndarray:
    from concourse.bass_utils import run_bass_kernel_spmd

    if "nc" not in _CACHE:
        _CACHE["nc"] = _build()
    nc = _CACHE["nc"]

    ft = np.ascontiguousarray(np.asarray(feat_trainable, dtype=np.float32).reshape(B * T, N, C))
    fc = np.ascontiguousarray(np.asarray(feat_criterion, dtype=np.float32).reshape(B * T, N, C))
    gm = np.ascontiguousarray(np.asarray(grp_masks, dtype=np.float32).reshape(B * T, S, N))

    in_maps = []
    for c in range(N_CORES):
        fr = slice(c * FRAMES_PER_CORE, (c + 1) * FRAMES_PER_CORE)
        in_maps.append({
            "ft": np.ascontiguousarray(ft[fr]),
            "fc": np.ascontiguousarray(fc[fr]),
            "gm": np.ascontiguousarray(gm[fr]),
        })

    import time
    last_err = None
    for attempt in range(4):
        try:
            res = run_bass_kernel_spmd(nc, in_maps, list(range(N_CORES)))
            break
        except Exception as e:  # wedged-device recovery: wait and retry
            last_err = e
            time.sleep(20 + 25 * attempt)
    else:
        raise last_err
    total = np.float64(0.0)
    for c in range(N_CORES):
        o = np.asarray(res.results[c]["out"], dtype=np.float64)
        total += POST * o[:, :NT].sum()
        st = o[:, 16:].reshape(128, GROUPS_PER_CORE, NT, 2).sum(axis=-1)
        total -= 2.0 * np.log(st).sum()
    loss = SCALE * total / (G * M) / 2.0
    return np.asarray(loss, dtype=np.float32)


if __name__ == "__main__":
    nc = _build()
    print("build OK")


# revision 4
# speedup vs baseline: 1.5387x; 1.0252x over previous
"""DECConsLoss Trainium2 kernel v3: 8-core data-parallel over groups.

Reference (per group g of G=32, M=2048 tokens, C=512):
  ft_n, fc_n = l2norm(ft), l2norm(fc);  grp[m] = argmax_s masks
  logits = ft_n @ fc_n^T / 0.1;  lse[m] = logsumexp(logits[m,:])
  semi[m] = scale*(mean_{n: grp[n]==grp[m]} logits[m,n] - lse)
  pos[m]  = scale*(logits[m,m] - lse);  loss = mean(semi+pos)/2

v3 pipeline (all primitives hardware-validated):
  - SWDGE casting DMA loads fp32 DRAM -> bf16 SBUF token-major [128,(j,c)]
  - DMA XBAR transposes bf16 SBUF->SBUF -> [128,(j,k,m)] ("alt" layout)
  - ssq via plain bf16 Gram matmuls; diag extracted by tensor_tensor
    against a 4-wide identity + segmented tensor_reduce
  - 16*rsqrt(ssq) via Newton iteration on gpsimd (no ACT table thrash)
  - both tensors scaled to fp8 with broadcast matrices A (j-major out,
    gating per chunk) / B (k-major out, gating per n-half); exp scale is
    then the constant 10/256
  - main loop runs n-half-outer so the first 16 exps only need fc's
    first-half chain; ft's full chain is shorter
  - per-i diag from fp8 DR cross-Grams; masked means via Q/P side-GEMMs
  - exp in-place on PSUM with accum_out row-sums; ln(sums) on the host
Host: loss = SCALE*((10/256)*sum(acc0) - 2*sum(ln(stot))) / (G*M) / 2.
"""

import sys
import numpy as np

for p in ("/opt/trn_rl_repo", "/opt/trn_rl_repo/concourse", "/opt/pypackages"):
    if p not in sys.path:
        sys.path.insert(0, p)

GF = 2
S = 16
N = 1024
C = 512
B, T = 8, 8
G = (B * T) // GF            # 32 groups total
M = GF * N                   # 2048 tokens per group
N_CORES = 8
GROUPS_PER_CORE = G // N_CORES   # 4
FRAMES_PER_CORE = GROUPS_PER_CORE * GF  # 8
TEMP = 0.1
BASE_TEMP = 0.07
INV_TEMP = 1.0 / TEMP
SCALE = -(TEMP / BASE_TEMP)
FS = 16.0                    # fp8 pre-scale on both normalized tensors
POST = INV_TEMP / (FS * FS)  # 10/256: logits = POST * H

NT = M // 128                # 16 token tiles per group
KC = C // 128                # 4 contraction chunks

_CACHE = {}


def _build():
    import math
    import concourse.mybir as mybir
    from concourse import bacc
    from concourse import masks
    from concourse import bass_isa
    from concourse.tile import TileContext

    dt = mybir.dt
    Alu = mybir.AluOpType
    Act = mybir.ActivationFunctionType
    DR = mybir.MatmulPerfMode.DoubleRow

    nc = bacc.Bacc()
    ft_d = nc.declare_dram_parameter("ft", [FRAMES_PER_CORE, N, C], dt.float32, isOutput=False)
    fc_d = nc.declare_dram_parameter("fc", [FRAMES_PER_CORE, N, C], dt.float32, isOutput=False)
    gm_d = nc.declare_dram_parameter("gm", [FRAMES_PER_CORE, S, N], dt.float32, isOutput=False)
    out_d = nc.declare_dram_parameter("out", [128, 16 + 32 * GROUPS_PER_CORE], dt.float32, isOutput=True)

    f8 = dt.float8e4
    bf = dt.bfloat16

    with TileContext(nc) as tc:
        with (
            tc.tile_pool(name="consts", bufs=1) as consts,
            tc.tile_pool(name="tok_pool", bufs=2) as tok_pool,
            tc.tile_pool(name="tT_pool", bufs=2) as tT_pool,
            tc.tile_pool(name="n8_pool", bufs=2) as n8_pool,
            tc.tile_pool(name="ab_pool", bufs=1) as ab_pool,
            tc.tile_pool(name="grp_pool", bufs=2) as grp_pool,
            tc.tile_pool(name="stat_pool", bufs=2) as stat_pool,
            tc.tile_pool(name="scr_pool", bufs=2) as scr_pool,
            tc.tile_pool(name="q_pool", bufs=2) as q_pool,
            tc.tile_pool(name="acc_pool", bufs=1) as acc_pool,
            tc.tile_pool(name="sm_psum", bufs=2, space="PSUM") as sm_psum,
            tc.tile_pool(name="lg_psum", bufs=3, space="PSUM") as lg_psum,
        ):
            # ---- constants ----
            id128_bf = consts.tile([128, 128], bf)
            id128_f32 = consts.tile([128, 128], dt.float32)
            id16_bf = consts.tile([S, S], bf)
            id16_f32 = consts.tile([S, S], dt.float32)
            for t in (id128_bf, id128_f32, id16_bf, id16_f32):
                nc.vector.memset(t[:], 0.0)
                masks.make_identity(nc, t[:], nomemset=True)
            id4_bf = consts.tile([128, 512], bf)
            for q in range(4):
                nc.gpsimd.tensor_copy(id4_bf[:, q * 128 : (q + 1) * 128], id128_bf[:])
            # E8[:, jj*128:(jj+1)*128] = row-jj selector over 8 partitions
            E8 = consts.tile([8, 8 * 128], bf)
            nc.gpsimd.memset(E8[:], 0.0)
            nc.gpsimd.affine_select(
                out=E8[:], in_=E8[:],
                compare_op=mybir.AluOpType.not_equal, fill=1.0, base=0,
                pattern=[[-1, 8], [0, 128]], channel_multiplier=1,
            )

            acc = acc_pool.tile([128, 16 + 32 * GROUPS_PER_CORE], dt.float32)
            nc.vector.memset(acc[:, 0:16], 0.0)

            w0 = FS / math.sqrt(512.0)

            def newton16(ssq, dst, cols):
                """dst[:, cols] = FS*rsqrt(max(ssq,eps)) via 3 Newton steps."""
                sq, dv = ssq[:, cols], dst[:, cols]
                wscr = stat_pool.tile([128, 8], dt.float32, tag="wscr")
                nc.gpsimd.tensor_scalar(
                    out=sq, in0=sq, scalar1=1e-12, scalar2=1.0 / (FS * FS),
                    op0=Alu.max, op1=Alu.mult,
                )
                nc.gpsimd.tensor_scalar(
                    out=dv, in0=sq, scalar1=-0.5 * w0 ** 3, scalar2=1.5 * w0,
                    op0=Alu.mult, op1=Alu.add,
                )
                for _ in range(2):
                    nc.gpsimd.tensor_tensor(out=wscr[:], in0=dv, in1=dv, op=Alu.mult)
                    nc.gpsimd.tensor_tensor(out=wscr[:], in0=wscr[:], in1=sq, op=Alu.mult)
                    nc.gpsimd.tensor_scalar(
                        out=wscr[:], in0=wscr[:], scalar1=-0.5, scalar2=1.5,
                        op0=Alu.mult, op1=Alu.add,
                    )
                    nc.gpsimd.tensor_tensor(out=dv, in0=dv, in1=wscr[:], op=Alu.mult)

            def bcast_half(src, mat, h, g, nm):
                """mat[:, h*1024:(h+1)*1024] <- broadcast rows of src[:, h*8:(h+1)*8]."""
                tps = sm_psum.tile([8, 128], dt.float32, tag="sm", name=f"{nm}tp{g}_{h}")
                nc.tensor.transpose(tps[:], src[:, h * 8 : (h + 1) * 8], id128_f32[:])
                row_sb = q_pool.tile([8, 128], bf, tag=f"{nm}row")
                nc.vector.tensor_copy(row_sb[:], tps[:])
                for half_q in range(2):
                    bps = sm_psum.tile([128, 512], dt.float32, tag="sm", name=f"{nm}ps{g}_{h}_{half_q}")
                    for jj in range(4):
                        j = half_q * 4 + jj
                        nc.tensor.matmul(
                            bps[:, jj * 128 : (jj + 1) * 128],
                            E8[:, j * 128 : (j + 1) * 128], row_sb[:],
                            start=True, stop=True,
                        )
                    nc.vector.tensor_copy(
                        mat[:, h * 1024 + half_q * 512 : h * 1024 + (half_q + 1) * 512], bps[:]
                    )

            def gram_bank(T_bf, ssq, i0, g, nm, src3=None):
                gm = sm_psum.tile([128, 512], dt.float32, tag="sm", name=f"{nm}{g}_{i0}")
                for di in range(4):
                    i = i0 + di
                    if src3 is None:
                        for k in range(KC):
                            sl = slice(i * 512 + k * 128, i * 512 + (k + 1) * 128)
                            nc.tensor.matmul(
                                gm[:, di * 128 : (di + 1) * 128],
                                T_bf[:, sl], T_bf[:, sl],
                                start=(k == 0), stop=(k == KC - 1),
                            )
                    else:
                        lhs3, rhs3 = src3
                        for kp in (0, 2):
                            nc.tensor.matmul(
                                gm[:, di * 128 : (di + 1) * 128],
                                lhs3[:, kp : kp + 2, i * 128 : (i + 1) * 128],
                                rhs3[:, kp : kp + 2, i * 128 : (i + 1) * 128],
                                start=(kp == 0), stop=(kp == 2), perf_mode=DR,
                            )
                scr = scr_pool.tile([128, 512], bf, tag="scr")
                nc.vector.tensor_tensor(out=scr[:], in0=gm[:], in1=id4_bf[:], op=Alu.mult)
                nc.vector.tensor_reduce(
                    out=ssq[:, i0 : i0 + 4],
                    in_=scr.rearrange("p (q m) -> p q m", q=4),
                    axis=mybir.AxisListType.X, op=Alu.add,
                )

            for g in range(GROUPS_PER_CORE):
                # ============ phase B: fc chain ============
                fc_bf = tok_pool.tile([128, NT * C], bf, tag="fcbf", name=f"fcbf{g}")
                fcT = tT_pool.tile([128, NT * C], bf, tag="fcT", name=f"fcT{g}")
                ssq_fc = stat_pool.tile([128, NT], dt.float32, tag="ssq_fc")
                bprime = stat_pool.tile([128, NT], dt.float32, tag="bprime")
                B_sb = ab_pool.tile([128, M], bf, tag="Bmat")
                fcn = n8_pool.tile([128, KC * M], f8, tag="fcn", name=f"fcn{g}")
                nc.gpsimd.dma_start(
                    out=fc_bf.rearrange("p (j c) -> p j c", j=NT),
                    in_=fc_d[2 * g : 2 * g + 2].rearrange("f (jj p) c -> p (f jj) c", p=128),
                )
                nc.sync.dma_start_transpose(
                    out=fcT.rearrange("p (q m) -> p q m", m=128), in_=fc_bf[:]
                )
                for i0 in range(0, NT, 4):
                    gram_bank(fcT, ssq_fc, i0, g, "cg")
                # ============ phase A: ft full chain (gates lhsT side) ============
                ft_bf = tok_pool.tile([128, NT * C], bf, tag="ftbf", name=f"ftbf{g}")
                nc.gpsimd.dma_start(
                    out=ft_bf.rearrange("p (j c) -> p j c", j=NT),
                    in_=ft_d[2 * g : 2 * g + 2].rearrange("f (jj p) c -> p (f jj) c", p=128),
                )
                ftT = tT_pool.tile([128, NT * C], bf, tag="ftT", name=f"ftT{g}")
                nc.sync.dma_start_transpose(
                    out=ftT.rearrange("p (q m) -> p q m", m=128), in_=ft_bf[:]
                )
                ssq_ft = stat_pool.tile([128, NT], dt.float32, tag="ssq_ft")
                for i0 in range(0, NT, 4):
                    gram_bank(ftT, ssq_ft, i0, g, "fg")
                for h in range(2):
                    newton16(ssq_fc, bprime, slice(h * 8, (h + 1) * 8))
                    bcast_half(bprime, B_sb, h, g, "B")
                # fcn k-major: out col = k*2048 + n
                for h in range(2):
                    for k in range(KC):
                        eng = nc.vector if k % 2 == 0 else nc.gpsimd
                        eng.tensor_tensor(
                            out=fcn[:, k * M + h * 1024 : k * M + (h + 1) * 1024]
                                .rearrange("p (j m) -> p j m", m=128),
                            in0=fcT.rearrange("p (j km) -> p j km", j=NT)
                                [:, h * 8 : (h + 1) * 8, k * 128 : (k + 1) * 128],
                            in1=B_sb[:, h * 1024 : (h + 1) * 1024]
                                .rearrange("p (j m) -> p j m", m=128),
                            op=Alu.mult,
                        )
                fcn3 = fcn.rearrange("p (k n) -> p k n", k=KC)

                ahat = stat_pool.tile([128, NT], dt.float32, tag="ahat")
                for h in range(2):
                    newton16(ssq_ft, ahat, slice(h * 8, (h + 1) * 8))
                A_sb = ab_pool.tile([128, M], bf, tag="Amat")
                for h in range(2):
                    bcast_half(ahat, A_sb, h, g, "A")
                # ftn in alt (j-major) layout: col = j*512 + k*128 + m
                ftn = n8_pool.tile([128, KC * M], f8, tag="ftn", name=f"ftn{g}")
                for c in range(4):
                    for k in range(KC):
                        eng = nc.vector if k % 2 == 0 else nc.gpsimd
                        eng.tensor_tensor(
                            out=ftn.rearrange("p (j km) -> p j km", j=NT)
                                [:, c * 4 : (c + 1) * 4, k * 128 : (k + 1) * 128],
                            in0=ftT.rearrange("p (j km) -> p j km", j=NT)
                                [:, c * 4 : (c + 1) * 4, k * 128 : (k + 1) * 128],
                            in1=A_sb[:, c * 512 : (c + 1) * 512]
                                .rearrange("p (j m) -> p j m", m=128),
                            op=Alu.mult,
                        )

                # ============ main: logits GEMM (DR) + exp, n-half outer ============
                stot2 = stat_pool.tile([128, 2 * NT], dt.float32, tag="stot2")
                for i in range(NT):
                    for h in range(2):
                        lg = lg_psum.tile([128, 1024], dt.float32, tag="lg", name=f"lg{g}_{i}_{h}")
                        for nb in range(2):
                            blk = lg[:, nb * 512 : (nb + 1) * 512]
                            ncol = (2 * h + nb) * 512
                            for kp in (0, 2):
                                nc.tensor.matmul(
                                    blk,
                                    ftn[:, i * 512 + kp * 128 : i * 512 + (kp + 2) * 128]
                                        .rearrange("p (k m) -> p k m", k=2),
                                    fcn3[:, kp : kp + 2, ncol : ncol + 512],
                                    start=(kp == 0), stop=(kp == 2), perf_mode=DR,
                                )
                        nc.scalar.activation(
                            lg[:], lg[:], Act.Exp, scale=POST,
                            accum_out=stot2[:, 2 * i + h : 2 * i + h + 1],
                        )

                # ============ phase D: diag cross-Grams, Q, pp/mavg ============
                # grp-mask work is independent; emitted here so its engine use
                # overlaps the ft/fc chains above.
                grp_sb = grp_pool.tile([S, M], dt.float32, tag="grp_sb")
                nc.sync.dma_start(
                    out=grp_sb.rearrange("s (f n) -> s f n", f=GF),
                    in_=gm_d[2 * g : 2 * g + 2].rearrange("f s n -> s f n"),
                )
                tpg = sm_psum.tile([128, NT * S], dt.float32, tag="sm", name=f"tpg{g}")
                for j in range(NT):
                    nc.tensor.transpose(
                        tpg[:, j * S : (j + 1) * S], grp_sb[:, j * 128 : (j + 1) * 128], id16_f32[:]
                    )
                grpT = grp_pool.tile([128, NT * S], dt.float32, tag="grpT")
                nc.vector.tensor_copy(grpT[:], tpg[:])
                rowmax = stat_pool.tile([128, NT], dt.float32, tag="rowmax")
                nc.vector.tensor_reduce(
                    out=rowmax[:],
                    in_=grpT.rearrange("p (j s) -> p j s", j=NT),
                    axis=mybir.AxisListType.X, op=Alu.max,
                )
                oh_f32 = grp_pool.tile([128, NT * S], dt.float32, tag="oh_f32")
                oh_w = grp_pool.tile([128, NT * S], dt.float32, tag="oh_w")
                ohsum = stat_pool.tile([128, S], dt.float32, tag="ohsum")
                cntb = stat_pool.tile([128, S], dt.float32, tag="cntb")
                for j in range(NT):
                    sl = slice(j * S, (j + 1) * S)
                    nc.gpsimd.tensor_scalar(
                        out=oh_f32[:, sl], in0=grpT[:, sl],
                        scalar1=rowmax[:, j : j + 1], scalar2=None, op0=Alu.is_equal,
                    )
                    if j == 0:
                        nc.gpsimd.tensor_copy(ohsum[:], oh_f32[:, sl])
                    else:
                        nc.gpsimd.tensor_tensor(out=ohsum[:], in0=ohsum[:], in1=oh_f32[:, sl], op=Alu.add)
                nc.gpsimd.partition_all_reduce(
                    out_ap=cntb[:], in_ap=ohsum[:], channels=128, reduce_op=bass_isa.ReduceOp.add,
                )
                nc.gpsimd.tensor_scalar(out=cntb[:], in0=cntb[:], scalar1=1.0, scalar2=None, op0=Alu.max)
                nc.vector.reciprocal(out=cntb[:], in_=cntb[:])
                for j in range(NT):
                    sl = slice(j * S, (j + 1) * S)
                    nc.gpsimd.tensor_tensor(out=oh_w[:, sl], in0=oh_f32[:, sl], in1=cntb[:], op=Alu.mult)

                # cross-Gram diag (ftn alt-layout lhsT, fcn k-major rhs)
                diagF = stat_pool.tile([128, NT], dt.float32, tag="diagF")
                for i0 in range(0, NT, 4):
                    gmx = sm_psum.tile([128, 512], dt.float32, tag="sm", name=f"xg{g}_{i0}")
                    for di in range(4):
                        i = i0 + di
                        for kp in (0, 2):
                            nc.tensor.matmul(
                                gmx[:, di * 128 : (di + 1) * 128],
                                ftn[:, i * 512 + kp * 128 : i * 512 + (kp + 2) * 128]
                                    .rearrange("p (k m) -> p k m", k=2),
                                fcn3[:, kp : kp + 2, i * 128 : (i + 1) * 128],
                                start=(kp == 0), stop=(kp == 2), perf_mode=DR,
                            )
                    scr = scr_pool.tile([128, 512], bf, tag="scr")
                    nc.vector.tensor_tensor(out=scr[:], in0=gmx[:], in1=id4_bf[:], op=Alu.mult)
                    nc.vector.tensor_reduce(
                        out=diagF[:, i0 : i0 + 4],
                        in_=scr.rearrange("p (q m) -> p q m", q=4),
                        axis=mybir.AxisListType.X, op=Alu.add,
                    )

                oh_b = grp_pool.tile([128, NT * S], bf, tag="oh_b")
                for j in range(NT):
                    sl = slice(j * S, (j + 1) * S)
                    nc.gpsimd.tensor_scalar(
                        out=oh_b[:, sl], in0=oh_f32[:, sl],
                        scalar1=bprime[:, j : j + 1], scalar2=None, op0=Alu.mult,
                    )
                qq = sm_psum.tile([S, C], dt.float32, tag="sm", name=f"qq{g}")
                for j in range(NT):
                    nc.tensor.matmul(
                        qq[:], oh_b[:, j * S : (j + 1) * S], fc_bf[:, j * C : (j + 1) * C],
                        start=(j == 0), stop=(j == NT - 1),
                    )
                q_sb = q_pool.tile([S, C], bf, tag="q_sb")
                nc.vector.tensor_copy(q_sb[:], qq[:])
                qtp = sm_psum.tile([128, KC * S], bf, tag="sm", name=f"qtp{g}")
                for k in range(KC):
                    nc.tensor.transpose(
                        qtp[:, k * S : (k + 1) * S], q_sb[:, k * 128 : (k + 1) * 128], id16_bf[:]
                    )
                qt_sb = q_pool.tile([128, KC * S], bf, tag="qt_sb")
                nc.vector.tensor_copy(qt_sb[:], qtp[:])

                mavg = stat_pool.tile([128, NT], dt.float32, tag="mavg")
                for i0 in range(0, NT, 4):
                    ppt = sm_psum.tile([128, 4 * S], dt.float32, tag="sm", name=f"pp{g}_{i0}")
                    for di in range(4):
                        i = i0 + di
                        for k in range(KC):
                            nc.tensor.matmul(
                                ppt[:, di * S : (di + 1) * S],
                                ftn[:, i * 512 + k * 128 : i * 512 + (k + 1) * 128],
                                qt_sb[:, k * S : (k + 1) * S],
                                start=(k == 0), stop=(k == KC - 1),
                            )
                    pscr = scr_pool.tile([128, 4 * S], dt.float32, tag="pscr")
                    nc.vector.tensor_tensor(
                        out=pscr[:], in0=ppt[:], in1=oh_w[:, i0 * S : (i0 + 4) * S], op=Alu.mult
                    )
                    nc.vector.tensor_reduce(
                        out=mavg[:, i0 : i0 + 4],
                        in_=pscr.rearrange("p (q s) -> p q s", q=4),
                        axis=mybir.AxisListType.X, op=Alu.add,
                    )

                # ============ finals (lse ln'd host-side from raw stot2) ============
                tsum = stat_pool.tile([128, NT], dt.float32, tag="tsum")
                nc.gpsimd.tensor_tensor(out=tsum[:], in0=diagF[:], in1=mavg[:], op=Alu.add)
                nc.gpsimd.tensor_tensor(out=acc[:, 0:NT], in0=acc[:, 0:NT], in1=tsum[:], op=Alu.add)
                nc.gpsimd.tensor_copy(acc[:, 16 + 32 * g : 16 + 32 * (g + 1)], stot2[:])

            nc.sync.dma_start(out=out_d[:, :], in_=acc[:])

    nc.compile()
    return nc


def kernel(feat_trainable: np.ndarray, feat_criterion: np.ndarray, grp_masks: np.ndarray) -> np.ndarray:
    from concourse.bass_utils import run_bass_kernel_spmd

    if "nc" not in _CACHE:
        _CACHE["nc"] = _build()
    nc = _CACHE["nc"]

    ft = np.ascontiguousarray(np.asarray(feat_trainable, dtype=np.float32).reshape(B * T, N, C))
    fc = np.ascontiguousarray(np.asarray(feat_criterion, dtype=np.float32).reshape(B * T, N, C))
    gm = np.ascontiguousarray(np.asarray(grp_masks, dtype=np.float32).reshape(B * T, S, N))

    in_maps = []
    for c in range(N_CORES):
        fr = slice(c * FRAMES_PER_CORE, (c + 1) * FRAMES_PER_CORE)
        in_maps.append({
            "ft": np.ascontiguousarray(ft[fr]),
            "fc": np.ascontiguousarray(fc[fr]),
            "gm": np.ascontiguousarray(gm[fr]),
        })

    import time
    last_err = None
    for attempt in range(4):
        try:
            res = run_bass_kernel_spmd(nc, in_maps, list(range(N_CORES)))
            break
        except Exception as e:  # wedged-device recovery: wait and retry
            last_err = e
            time.sleep(20 + 25 * attempt)
    else:
        raise last_err
    total = np.float64(0.0)
    for c in range(N_CORES):
        o = np.asarray(res.results[c]["out"], dtype=np.float64)
        total += POST * o[:, :NT].sum()
        st = o[:, 16:].reshape(128, GROUPS_PER_CORE, NT, 2).sum(axis=-1)
        total -= 2.0 * np.log(st).sum()
    loss = SCALE * total / (G * M) / 2.0
    return np.asarray(loss, dtype=np.float32)


if __name__ == "__main__":
    nc = _build()
    print("build OK")


# revision 5
# speedup vs baseline: 1.9098x; 1.2412x over previous
"""DECConsLoss Trainium2 kernel v3: 8-core data-parallel over groups.

Reference (per group g of G=32, M=2048 tokens, C=512):
  ft_n, fc_n = l2norm(ft), l2norm(fc);  grp[m] = argmax_s masks
  logits = ft_n @ fc_n^T / 0.1;  lse[m] = logsumexp(logits[m,:])
  semi[m] = scale*(mean_{n: grp[n]==grp[m]} logits[m,n] - lse)
  pos[m]  = scale*(logits[m,m] - lse);  loss = mean(semi+pos)/2

v3 pipeline (all primitives hardware-validated):
  - SWDGE casting DMA loads fp32 DRAM -> bf16 SBUF token-major [128,(j,c)]
  - DMA XBAR transposes bf16 SBUF->SBUF -> [128,(j,k,m)] ("alt" layout)
  - ssq via plain bf16 Gram matmuls; diag extracted by tensor_tensor
    against a 4-wide identity + segmented tensor_reduce
  - 16*rsqrt(ssq) via Newton iteration on gpsimd (no ACT table thrash)
  - both tensors scaled to fp8 with broadcast matrices A (j-major out,
    gating per chunk) / B (k-major out, gating per n-half); exp scale is
    then the constant 10/256
  - main loop runs n-half-outer so the first 16 exps only need fc's
    first-half chain; ft's full chain is shorter
  - per-i diag from fp8 DR cross-Grams; masked means via Q/P side-GEMMs
  - exp in-place on PSUM with accum_out row-sums; ln(sums) on the host
Host: loss = SCALE*((10/256)*sum(acc0) - 2*sum(ln(stot))) / (G*M) / 2.
"""

import sys
import numpy as np

for p in ("/opt/trn_rl_repo", "/opt/trn_rl_repo/concourse", "/opt/pypackages"):
    if p not in sys.path:
        sys.path.insert(0, p)

GF = 2
S = 16
N = 1024
C = 512
B, T = 8, 8
G = (B * T) // GF            # 32 groups total
M = GF * N                   # 2048 tokens per group
N_CORES = 8
GROUPS_PER_CORE = G // N_CORES   # 4
FRAMES_PER_CORE = GROUPS_PER_CORE * GF  # 8
TEMP = 0.1
BASE_TEMP = 0.07
INV_TEMP = 1.0 / TEMP
SCALE = -(TEMP / BASE_TEMP)
FS = 16.0                    # fp8 pre-scale on both normalized tensors
POST = INV_TEMP / (FS * FS)  # 10/256: logits = POST * H

NT = M // 128                # 16 token tiles per group
KC = C // 128                # 4 contraction chunks

_CACHE = {}


def _build():
    import math
    import concourse.mybir as mybir
    from concourse import bacc
    from concourse import masks
    from concourse import bass_isa
    from concourse.tile import TileContext

    dt = mybir.dt
    Alu = mybir.AluOpType
    Act = mybir.ActivationFunctionType
    DR = mybir.MatmulPerfMode.DoubleRow

    nc = bacc.Bacc()
    ft_d = nc.declare_dram_parameter("ft", [FRAMES_PER_CORE, N, C], dt.bfloat16, isOutput=False)
    fc_d = nc.declare_dram_parameter("fc", [FRAMES_PER_CORE, N, C], dt.bfloat16, isOutput=False)
    gm_d = nc.declare_dram_parameter("gm", [FRAMES_PER_CORE, S, N], dt.float32, isOutput=False)
    out_d = nc.declare_dram_parameter("out", [128, 16 + 32 * GROUPS_PER_CORE], dt.float32, isOutput=True)

    f8 = dt.float8e4
    bf = dt.bfloat16

    with TileContext(nc) as tc:
        with (
            tc.tile_pool(name="consts", bufs=1) as consts,
            tc.tile_pool(name="tok_pool", bufs=2) as tok_pool,
            tc.tile_pool(name="tT_pool", bufs=2) as tT_pool,
            tc.tile_pool(name="n8_pool", bufs=2) as n8_pool,
            tc.tile_pool(name="ab_pool", bufs=1) as ab_pool,
            tc.tile_pool(name="grp_pool", bufs=2) as grp_pool,
            tc.tile_pool(name="stat_pool", bufs=2) as stat_pool,
            tc.tile_pool(name="scr_pool", bufs=2) as scr_pool,
            tc.tile_pool(name="q_pool", bufs=2) as q_pool,
            tc.tile_pool(name="acc_pool", bufs=1) as acc_pool,
            tc.tile_pool(name="sm_psum", bufs=2, space="PSUM") as sm_psum,
            tc.tile_pool(name="lg_psum", bufs=3, space="PSUM") as lg_psum,
        ):
            # ---- constants ----
            id128_bf = consts.tile([128, 128], bf)
            id128_f32 = consts.tile([128, 128], dt.float32)
            id16_bf = consts.tile([S, S], bf)
            id16_f32 = consts.tile([S, S], dt.float32)
            for t in (id128_bf, id128_f32, id16_bf, id16_f32):
                nc.vector.memset(t[:], 0.0)
                masks.make_identity(nc, t[:], nomemset=True)
            id4_bf = consts.tile([128, 512], bf)
            for q in range(4):
                nc.gpsimd.tensor_copy(id4_bf[:, q * 128 : (q + 1) * 128], id128_bf[:])
            # E8[:, jj*128:(jj+1)*128] = row-jj selector over 8 partitions
            E8 = consts.tile([8, 8 * 128], bf)
            nc.gpsimd.memset(E8[:], 0.0)
            nc.gpsimd.affine_select(
                out=E8[:], in_=E8[:],
                compare_op=mybir.AluOpType.not_equal, fill=1.0, base=0,
                pattern=[[-1, 8], [0, 128]], channel_multiplier=1,
            )

            acc = acc_pool.tile([128, 16 + 32 * GROUPS_PER_CORE], dt.float32)
            nc.vector.memset(acc[:, 0:16], 0.0)

            w0 = FS / math.sqrt(512.0)

            def newton16(ssq, dst, cols):
                """dst[:, cols] = FS*rsqrt(max(ssq,eps)) via 3 Newton steps."""
                sq, dv = ssq[:, cols], dst[:, cols]
                wscr = stat_pool.tile([128, 8], dt.float32, tag="wscr")
                nc.gpsimd.tensor_scalar(
                    out=sq, in0=sq, scalar1=1e-12, scalar2=1.0 / (FS * FS),
                    op0=Alu.max, op1=Alu.mult,
                )
                nc.gpsimd.tensor_scalar(
                    out=dv, in0=sq, scalar1=-0.5 * w0 ** 3, scalar2=1.5 * w0,
                    op0=Alu.mult, op1=Alu.add,
                )
                for _ in range(2):
                    nc.gpsimd.tensor_tensor(out=wscr[:], in0=dv, in1=dv, op=Alu.mult)
                    nc.gpsimd.tensor_tensor(out=wscr[:], in0=wscr[:], in1=sq, op=Alu.mult)
                    nc.gpsimd.tensor_scalar(
                        out=wscr[:], in0=wscr[:], scalar1=-0.5, scalar2=1.5,
                        op0=Alu.mult, op1=Alu.add,
                    )
                    nc.gpsimd.tensor_tensor(out=dv, in0=dv, in1=wscr[:], op=Alu.mult)

            def bcast_half(src, mat, h, g, nm):
                """mat[:, h*1024:(h+1)*1024] <- broadcast rows of src[:, h*8:(h+1)*8]."""
                tps = sm_psum.tile([8, 128], dt.float32, tag="sm", name=f"{nm}tp{g}_{h}")
                nc.tensor.transpose(tps[:], src[:, h * 8 : (h + 1) * 8], id128_f32[:])
                row_sb = q_pool.tile([8, 128], bf, tag=f"{nm}row")
                nc.vector.tensor_copy(row_sb[:], tps[:])
                for half_q in range(2):
                    bps = sm_psum.tile([128, 512], dt.float32, tag="sm", name=f"{nm}ps{g}_{h}_{half_q}")
                    for jj in range(4):
                        j = half_q * 4 + jj
                        nc.tensor.matmul(
                            bps[:, jj * 128 : (jj + 1) * 128],
                            E8[:, j * 128 : (j + 1) * 128], row_sb[:],
                            start=True, stop=True,
                        )
                    nc.vector.tensor_copy(
                        mat[:, h * 1024 + half_q * 512 : h * 1024 + (half_q + 1) * 512], bps[:]
                    )

            def gram_bank(T_bf, ssq, i0, g, nm, src3=None):
                gm = sm_psum.tile([128, 512], dt.float32, tag="sm", name=f"{nm}{g}_{i0}")
                for di in range(4):
                    i = i0 + di
                    if src3 is None:
                        for k in range(KC):
                            sl = slice(k * M + i * 128, k * M + (i + 1) * 128)
                            nc.tensor.matmul(
                                gm[:, di * 128 : (di + 1) * 128],
                                T_bf[:, sl], T_bf[:, sl],
                                start=(k == 0), stop=(k == KC - 1),
                            )
                    else:
                        lhs3, rhs3 = src3
                        for kp in (0, 2):
                            nc.tensor.matmul(
                                gm[:, di * 128 : (di + 1) * 128],
                                lhs3[:, kp : kp + 2, i * 128 : (i + 1) * 128],
                                rhs3[:, kp : kp + 2, i * 128 : (i + 1) * 128],
                                start=(kp == 0), stop=(kp == 2), perf_mode=DR,
                            )
                scr = scr_pool.tile([128, 512], bf, tag="scr")
                nc.vector.tensor_tensor(out=scr[:], in0=gm[:], in1=id4_bf[:], op=Alu.mult)
                nc.vector.tensor_reduce(
                    out=ssq[:, i0 : i0 + 4],
                    in_=scr.rearrange("p (q m) -> p q m", q=4),
                    axis=mybir.AxisListType.X, op=Alu.add,
                )

            for g in range(GROUPS_PER_CORE):
                # ============ phase B: fc chain ============
                fc_bf = tok_pool.tile([128, NT * C], bf, tag="fcbf", name=f"fcbf{g}")
                fcT = tT_pool.tile([128, NT * C], bf, tag="fcT", name=f"fcT{g}")
                ftT = tT_pool.tile([128, NT * C], bf, tag="ftT", name=f"ftT{g}")
                ssq_fc = stat_pool.tile([128, NT], dt.float32, tag="ssq_fc")
                ssq_ft = stat_pool.tile([128, NT], dt.float32, tag="ssq_ft")
                bprime = stat_pool.tile([128, NT], dt.float32, tag="bprime")
                B_sb = ab_pool.tile([128, M], bf, tag="Bmat")
                fcn = n8_pool.tile([128, KC * M], f8, tag="fcn", name=f"fcn{g}")
                # k-major transposes straight from DRAM: T[p, k*2048+m] = x[tok m, c=k*128+p]
                nc.sync.dma_start_transpose(
                    out=fcT.rearrange("p (q m) -> p q m", m=M),
                    in_=fc_d[2 * g : 2 * g + 2].rearrange("f n c -> (f n) c"),
                )
                nc.sync.dma_start_transpose(
                    out=ftT.rearrange("p (q m) -> p q m", m=M),
                    in_=ft_d[2 * g : 2 * g + 2].rearrange("f n c -> (f n) c"),
                )
                nc.sync.dma_start(
                    out=fc_bf.rearrange("p (j c) -> p j c", j=NT),
                    in_=fc_d[2 * g : 2 * g + 2].rearrange("f (jj p) c -> p (f jj) c", p=128),
                )
                for i0 in range(0, NT, 4):
                    gram_bank(fcT, ssq_fc, i0, g, "cg")
                for i0 in range(0, NT, 4):
                    gram_bank(ftT, ssq_ft, i0, g, "fg")
                for h in range(2):
                    newton16(ssq_fc, bprime, slice(h * 8, (h + 1) * 8))
                    bcast_half(bprime, B_sb, h, g, "B")
                # fcn k-major: out col = k*2048 + n
                for h in range(2):
                    for k in range(KC):
                        eng = nc.vector if k % 2 == 0 else nc.gpsimd
                        eng.tensor_tensor(
                            out=fcn[:, k * M + h * 1024 : k * M + (h + 1) * 1024],
                            in0=fcT[:, k * M + h * 1024 : k * M + (h + 1) * 1024],
                            in1=B_sb[:, h * 1024 : (h + 1) * 1024],
                            op=Alu.mult,
                        )
                fcn3 = fcn.rearrange("p (k n) -> p k n", k=KC)

                ahat = stat_pool.tile([128, NT], dt.float32, tag="ahat")
                for h in range(2):
                    newton16(ssq_ft, ahat, slice(h * 8, (h + 1) * 8))
                A_sb = ab_pool.tile([128, M], bf, tag="Amat")
                for h in range(2):
                    bcast_half(ahat, A_sb, h, g, "A")
                # ftn in alt (j-major) layout: col = j*512 + k*128 + m
                ftn = n8_pool.tile([128, KC * M], f8, tag="ftn", name=f"ftn{g}")
                ftn3 = ftn.rearrange("p (k n) -> p k n", k=KC)
                for c in range(4):
                    for k in range(KC):
                        eng = nc.vector if k % 2 == 0 else nc.gpsimd
                        eng.tensor_tensor(
                            out=ftn[:, k * M + c * 512 : k * M + (c + 1) * 512],
                            in0=ftT[:, k * M + c * 512 : k * M + (c + 1) * 512],
                            in1=A_sb[:, c * 512 : (c + 1) * 512],
                            op=Alu.mult,
                        )

                # ============ main: logits GEMM (DR) + exp, n-half outer ============
                stot2 = stat_pool.tile([128, 2 * NT], dt.float32, tag="stot2")
                for i in range(NT):
                    for h in range(2):
                        lg = lg_psum.tile([128, 1024], dt.float32, tag="lg", name=f"lg{g}_{i}_{h}")
                        for nb in range(2):
                            blk = lg[:, nb * 512 : (nb + 1) * 512]
                            ncol = (2 * h + nb) * 512
                            for kp in (0, 2):
                                nc.tensor.matmul(
                                    blk,
                                    ftn3[:, kp : kp + 2, i * 128 : (i + 1) * 128],
                                    fcn3[:, kp : kp + 2, ncol : ncol + 512],
                                    start=(kp == 0), stop=(kp == 2), perf_mode=DR,
                                )
                        nc.scalar.activation(
                            lg[:], lg[:], Act.Exp, scale=POST,
                            accum_out=stot2[:, 2 * i + h : 2 * i + h + 1],
                        )

                # ============ phase D: diag cross-Grams, Q, pp/mavg ============
                # grp-mask work is independent; emitted here so its engine use
                # overlaps the ft/fc chains above.
                grp_sb = grp_pool.tile([S, M], dt.float32, tag="grp_sb")
                nc.sync.dma_start(
                    out=grp_sb.rearrange("s (f n) -> s f n", f=GF),
                    in_=gm_d[2 * g : 2 * g + 2].rearrange("f s n -> s f n"),
                )
                tpg = sm_psum.tile([128, NT * S], dt.float32, tag="sm", name=f"tpg{g}")
                for j in range(NT):
                    nc.tensor.transpose(
                        tpg[:, j * S : (j + 1) * S], grp_sb[:, j * 128 : (j + 1) * 128], id16_f32[:]
                    )
                grpT = grp_pool.tile([128, NT * S], dt.float32, tag="grpT")
                nc.vector.tensor_copy(grpT[:], tpg[:])
                rowmax = stat_pool.tile([128, NT], dt.float32, tag="rowmax")
                nc.vector.tensor_reduce(
                    out=rowmax[:],
                    in_=grpT.rearrange("p (j s) -> p j s", j=NT),
                    axis=mybir.AxisListType.X, op=Alu.max,
                )
                oh_f32 = grp_pool.tile([128, NT * S], dt.float32, tag="oh_f32")
                oh_w = grp_pool.tile([128, NT * S], dt.float32, tag="oh_w")
                ohsum = stat_pool.tile([128, S], dt.float32, tag="ohsum")
                cntb = stat_pool.tile([128, S], dt.float32, tag="cntb")
                for j in range(NT):
                    sl = slice(j * S, (j + 1) * S)
                    nc.gpsimd.tensor_scalar(
                        out=oh_f32[:, sl], in0=grpT[:, sl],
                        scalar1=rowmax[:, j : j + 1], scalar2=None, op0=Alu.is_equal,
                    )
                    if j == 0:
                        nc.gpsimd.tensor_copy(ohsum[:], oh_f32[:, sl])
                    else:
                        nc.gpsimd.tensor_tensor(out=ohsum[:], in0=ohsum[:], in1=oh_f32[:, sl], op=Alu.add)
                nc.gpsimd.partition_all_reduce(
                    out_ap=cntb[:], in_ap=ohsum[:], channels=128, reduce_op=bass_isa.ReduceOp.add,
                )
                nc.gpsimd.tensor_scalar(out=cntb[:], in0=cntb[:], scalar1=1.0, scalar2=None, op0=Alu.max)
                nc.vector.reciprocal(out=cntb[:], in_=cntb[:])
                for j in range(NT):
                    sl = slice(j * S, (j + 1) * S)
                    nc.gpsimd.tensor_tensor(out=oh_w[:, sl], in0=oh_f32[:, sl], in1=cntb[:], op=Alu.mult)

                # cross-Gram diag (ftn alt-layout lhsT, fcn k-major rhs)
                diagF = stat_pool.tile([128, NT], dt.float32, tag="diagF")
                for i0 in range(0, NT, 4):
                    gmx = sm_psum.tile([128, 512], dt.float32, tag="sm", name=f"xg{g}_{i0}")
                    for di in range(4):
                        i = i0 + di
                        for kp in (0, 2):
                            nc.tensor.matmul(
                                gmx[:, di * 128 : (di + 1) * 128],
                                ftn3[:, kp : kp + 2, i * 128 : (i + 1) * 128],
                                fcn3[:, kp : kp + 2, i * 128 : (i + 1) * 128],
                                start=(kp == 0), stop=(kp == 2), perf_mode=DR,
                            )
                    scr = scr_pool.tile([128, 512], bf, tag="scr")
                    nc.vector.tensor_tensor(out=scr[:], in0=gmx[:], in1=id4_bf[:], op=Alu.mult)
                    nc.vector.tensor_reduce(
                        out=diagF[:, i0 : i0 + 4],
                        in_=scr.rearrange("p (q m) -> p q m", q=4),
                        axis=mybir.AxisListType.X, op=Alu.add,
                    )

                oh_b = grp_pool.tile([128, NT * S], bf, tag="oh_b")
                for j in range(NT):
                    sl = slice(j * S, (j + 1) * S)
                    nc.gpsimd.tensor_scalar(
                        out=oh_b[:, sl], in0=oh_f32[:, sl],
                        scalar1=bprime[:, j : j + 1], scalar2=None, op0=Alu.mult,
                    )
                qq = sm_psum.tile([S, C], dt.float32, tag="sm", name=f"qq{g}")
                for j in range(NT):
                    nc.tensor.matmul(
                        qq[:], oh_b[:, j * S : (j + 1) * S], fc_bf[:, j * C : (j + 1) * C],
                        start=(j == 0), stop=(j == NT - 1),
                    )
                q_sb = q_pool.tile([S, C], bf, tag="q_sb")
                nc.vector.tensor_copy(q_sb[:], qq[:])
                qtp = sm_psum.tile([128, KC * S], bf, tag="sm", name=f"qtp{g}")
                for k in range(KC):
                    nc.tensor.transpose(
                        qtp[:, k * S : (k + 1) * S], q_sb[:, k * 128 : (k + 1) * 128], id16_bf[:]
                    )
                qt_sb = q_pool.tile([128, KC * S], bf, tag="qt_sb")
                nc.vector.tensor_copy(qt_sb[:], qtp[:])

                mavg = stat_pool.tile([128, NT], dt.float32, tag="mavg")
                for i0 in range(0, NT, 4):
                    ppt = sm_psum.tile([128, 4 * S], dt.float32, tag="sm", name=f"pp{g}_{i0}")
                    for di in range(4):
                        i = i0 + di
                        for k in range(KC):
                            nc.tensor.matmul(
                                ppt[:, di * S : (di + 1) * S],
                                ftn[:, k * M + i * 128 : k * M + (i + 1) * 128],
                                qt_sb[:, k * S : (k + 1) * S],
                                start=(k == 0), stop=(k == KC - 1),
                            )
                    pscr = scr_pool.tile([128, 4 * S], dt.float32, tag="pscr")
                    nc.vector.tensor_tensor(
                        out=pscr[:], in0=ppt[:], in1=oh_w[:, i0 * S : (i0 + 4) * S], op=Alu.mult
                    )
                    nc.vector.tensor_reduce(
                        out=mavg[:, i0 : i0 + 4],
                        in_=pscr.rearrange("p (q s) -> p q s", q=4),
                        axis=mybir.AxisListType.X, op=Alu.add,
                    )

                # ============ finals (lse ln'd host-side from raw stot2) ============
                tsum = stat_pool.tile([128, NT], dt.float32, tag="tsum")
                nc.gpsimd.tensor_tensor(out=tsum[:], in0=diagF[:], in1=mavg[:], op=Alu.add)
                nc.gpsimd.tensor_tensor(out=acc[:, 0:NT], in0=acc[:, 0:NT], in1=tsum[:], op=Alu.add)
                nc.gpsimd.tensor_copy(acc[:, 16 + 32 * g : 16 + 32 * (g + 1)], stot2[:])

            nc.sync.dma_start(out=out_d[:, :], in_=acc[:])

    nc.compile()
    return nc


def kernel(feat_trainable: np.ndarray, feat_criterion: np.ndarray, grp_masks: np.ndarray) -> np.ndarray:
    from concourse.bass_utils import run_bass_kernel_spmd

    if "nc" not in _CACHE:
        _CACHE["nc"] = _build()
    nc = _CACHE["nc"]

    import ml_dtypes
    ft = np.ascontiguousarray(
        np.asarray(feat_trainable, dtype=np.float32).reshape(B * T, N, C).astype(ml_dtypes.bfloat16))
    fc = np.ascontiguousarray(
        np.asarray(feat_criterion, dtype=np.float32).reshape(B * T, N, C).astype(ml_dtypes.bfloat16))
    gm = np.ascontiguousarray(np.asarray(grp_masks, dtype=np.float32).reshape(B * T, S, N))

    in_maps = []
    for c in range(N_CORES):
        fr = slice(c * FRAMES_PER_CORE, (c + 1) * FRAMES_PER_CORE)
        in_maps.append({
            "ft": np.ascontiguousarray(ft[fr]),
            "fc": np.ascontiguousarray(fc[fr]),
            "gm": np.ascontiguousarray(gm[fr]),
        })

    import time
    last_err = None
    for attempt in range(4):
        try:
            res = run_bass_kernel_spmd(nc, in_maps, list(range(N_CORES)))
            break
        except Exception as e:  # wedged-device recovery: wait and retry
            last_err = e
            time.sleep(20 + 25 * attempt)
    else:
        raise last_err
    total = np.float64(0.0)
    for c in range(N_CORES):
        o = np.asarray(res.results[c]["out"], dtype=np.float64)
        total += POST * o[:, :NT].sum()
        st = o[:, 16:].reshape(128, GROUPS_PER_CORE, NT, 2).sum(axis=-1)
        total -= 2.0 * np.log(st).sum()
    loss = SCALE * total / (G * M) / 2.0
    return np.asarray(loss, dtype=np.float32)


if __name__ == "__main__":
    nc = _build()
    print("build OK")


# revision 6
# speedup vs baseline: 1.9227x; 1.0068x over previous
"""DECConsLoss Trainium2 kernel v3: 8-core data-parallel over groups.

Reference (per group g of G=32, M=2048 tokens, C=512):
  ft_n, fc_n = l2norm(ft), l2norm(fc);  grp[m] = argmax_s masks
  logits = ft_n @ fc_n^T / 0.1;  lse[m] = logsumexp(logits[m,:])
  semi[m] = scale*(mean_{n: grp[n]==grp[m]} logits[m,n] - lse)
  pos[m]  = scale*(logits[m,m] - lse);  loss = mean(semi+pos)/2

v3 pipeline (all primitives hardware-validated):
  - SWDGE casting DMA loads fp32 DRAM -> bf16 SBUF token-major [128,(j,c)]
  - DMA XBAR transposes bf16 SBUF->SBUF -> [128,(j,k,m)] ("alt" layout)
  - ssq via plain bf16 Gram matmuls; diag extracted by tensor_tensor
    against a 4-wide identity + segmented tensor_reduce
  - 16*rsqrt(ssq) via Newton iteration on gpsimd (no ACT table thrash)
  - both tensors scaled to fp8 with broadcast matrices A (j-major out,
    gating per chunk) / B (k-major out, gating per n-half); exp scale is
    then the constant 10/256
  - main loop runs n-half-outer so the first 16 exps only need fc's
    first-half chain; ft's full chain is shorter
  - per-i diag from fp8 DR cross-Grams; masked means via Q/P side-GEMMs
  - exp in-place on PSUM with accum_out row-sums; ln(sums) on the host
Host: loss = SCALE*((10/256)*sum(acc0) - 2*sum(ln(stot))) / (G*M) / 2.
"""

import sys
import numpy as np

for p in ("/opt/trn_rl_repo", "/opt/trn_rl_repo/concourse", "/opt/pypackages"):
    if p not in sys.path:
        sys.path.insert(0, p)

GF = 2
S = 16
N = 1024
C = 512
B, T = 8, 8
G = (B * T) // GF            # 32 groups total
M = GF * N                   # 2048 tokens per group
N_CORES = 8
GROUPS_PER_CORE = G // N_CORES   # 4
FRAMES_PER_CORE = GROUPS_PER_CORE * GF  # 8
TEMP = 0.1
BASE_TEMP = 0.07
INV_TEMP = 1.0 / TEMP
SCALE = -(TEMP / BASE_TEMP)
FS = 16.0                    # fp8 pre-scale on both normalized tensors
POST = INV_TEMP / (FS * FS)  # 10/256: logits = POST * H

NT = M // 128                # 16 token tiles per group
KC = C // 128                # 4 contraction chunks

_CACHE = {}


def _build():
    import math
    import concourse.mybir as mybir
    from concourse import bacc
    from concourse import masks
    from concourse import bass_isa
    from concourse.tile import TileContext

    dt = mybir.dt
    Alu = mybir.AluOpType
    Act = mybir.ActivationFunctionType
    DR = mybir.MatmulPerfMode.DoubleRow

    nc = bacc.Bacc()
    ft_d = nc.declare_dram_parameter("ft", [FRAMES_PER_CORE, N, C], dt.bfloat16, isOutput=False)
    fc_d = nc.declare_dram_parameter("fc", [FRAMES_PER_CORE, N, C], dt.bfloat16, isOutput=False)
    gm_d = nc.declare_dram_parameter("gm", [FRAMES_PER_CORE, S, N], dt.float32, isOutput=False)
    out_d = nc.declare_dram_parameter("out", [128, 16 + 32 * GROUPS_PER_CORE], dt.float32, isOutput=True)

    f8 = dt.float8e4
    bf = dt.bfloat16

    with TileContext(nc) as tc:
        with (
            tc.tile_pool(name="consts", bufs=1) as consts,
            tc.tile_pool(name="tok_pool", bufs=2) as tok_pool,
            tc.tile_pool(name="tT_pool", bufs=4) as tT_pool,
            tc.tile_pool(name="n8_pool", bufs=2) as n8_pool,
            tc.tile_pool(name="ab_pool", bufs=1) as ab_pool,
            tc.tile_pool(name="grp_pool", bufs=2) as grp_pool,
            tc.tile_pool(name="stat_pool", bufs=2) as stat_pool,
            tc.tile_pool(name="scr_pool", bufs=2) as scr_pool,
            tc.tile_pool(name="q_pool", bufs=2) as q_pool,
            tc.tile_pool(name="acc_pool", bufs=1) as acc_pool,
            tc.tile_pool(name="sm_psum", bufs=2, space="PSUM") as sm_psum,
            tc.tile_pool(name="lg_psum", bufs=3, space="PSUM") as lg_psum,
        ):
            # ---- constants ----
            id128_bf = consts.tile([128, 128], bf)
            id128_f32 = consts.tile([128, 128], dt.float32)
            id16_bf = consts.tile([S, S], bf)
            id16_f32 = consts.tile([S, S], dt.float32)
            for t in (id128_bf, id128_f32, id16_bf, id16_f32):
                nc.vector.memset(t[:], 0.0)
                masks.make_identity(nc, t[:], nomemset=True)
            id4_bf = consts.tile([128, 512], bf)
            for q in range(4):
                nc.gpsimd.tensor_copy(id4_bf[:, q * 128 : (q + 1) * 128], id128_bf[:])
            # E8[:, jj*128:(jj+1)*128] = row-jj selector over 8 partitions
            E8 = consts.tile([8, 8 * 128], bf)
            nc.gpsimd.memset(E8[:], 0.0)
            nc.gpsimd.affine_select(
                out=E8[:], in_=E8[:],
                compare_op=mybir.AluOpType.not_equal, fill=1.0, base=0,
                pattern=[[-1, 8], [0, 128]], channel_multiplier=1,
            )

            acc = acc_pool.tile([128, 16 + 32 * GROUPS_PER_CORE], dt.float32)
            nc.vector.memset(acc[:, 0:16], 0.0)

            w0 = FS / math.sqrt(512.0)

            def newton16(ssq, dst, cols):
                """dst[:, cols] = FS*rsqrt(max(ssq,eps)) via 3 Newton steps."""
                sq, dv = ssq[:, cols], dst[:, cols]
                wscr = stat_pool.tile([128, 8], dt.float32, tag="wscr")
                nc.gpsimd.tensor_scalar(
                    out=sq, in0=sq, scalar1=1e-12, scalar2=1.0 / (FS * FS),
                    op0=Alu.max, op1=Alu.mult,
                )
                nc.gpsimd.tensor_scalar(
                    out=dv, in0=sq, scalar1=-0.5 * w0 ** 3, scalar2=1.5 * w0,
                    op0=Alu.mult, op1=Alu.add,
                )
                for _ in range(2):
                    nc.gpsimd.tensor_tensor(out=wscr[:], in0=dv, in1=dv, op=Alu.mult)
                    nc.gpsimd.tensor_tensor(out=wscr[:], in0=wscr[:], in1=sq, op=Alu.mult)
                    nc.gpsimd.tensor_scalar(
                        out=wscr[:], in0=wscr[:], scalar1=-0.5, scalar2=1.5,
                        op0=Alu.mult, op1=Alu.add,
                    )
                    nc.gpsimd.tensor_tensor(out=dv, in0=dv, in1=wscr[:], op=Alu.mult)

            def bcast_half(src, mat, h, g, nm):
                """mat[:, h*1024:(h+1)*1024] <- broadcast rows of src[:, h*8:(h+1)*8]."""
                tps = sm_psum.tile([8, 128], dt.float32, tag="sm", name=f"{nm}tp{g}_{h}")
                nc.tensor.transpose(tps[:], src[:, h * 8 : (h + 1) * 8], id128_f32[:])
                row_sb = q_pool.tile([8, 128], bf, tag=f"{nm}row")
                nc.vector.tensor_copy(row_sb[:], tps[:])
                for half_q in range(2):
                    bps = sm_psum.tile([128, 512], dt.float32, tag="sm", name=f"{nm}ps{g}_{h}_{half_q}")
                    for jj in range(4):
                        j = half_q * 4 + jj
                        nc.tensor.matmul(
                            bps[:, jj * 128 : (jj + 1) * 128],
                            E8[:, j * 128 : (j + 1) * 128], row_sb[:],
                            start=True, stop=True,
                        )
                    nc.vector.tensor_copy(
                        mat[:, h * 1024 + half_q * 512 : h * 1024 + (half_q + 1) * 512], bps[:]
                    )

            def gram_bank(T_bf, ssq, i0, g, nm, src3=None):
                gm = sm_psum.tile([128, 512], dt.float32, tag="sm", name=f"{nm}{g}_{i0}")
                for di in range(4):
                    i = i0 + di
                    if src3 is None:
                        for k in range(KC):
                            sl = slice(k * 1024 + (i % 8) * 128, k * 1024 + (i % 8 + 1) * 128)
                            nc.tensor.matmul(
                                gm[:, di * 128 : (di + 1) * 128],
                                T_bf[i // 8][:, sl], T_bf[i // 8][:, sl],
                                start=(k == 0), stop=(k == KC - 1),
                            )
                    else:
                        lhs3, rhs3 = src3
                        for kp in (0, 2):
                            nc.tensor.matmul(
                                gm[:, di * 128 : (di + 1) * 128],
                                lhs3[:, kp : kp + 2, i * 128 : (i + 1) * 128],
                                rhs3[:, kp : kp + 2, i * 128 : (i + 1) * 128],
                                start=(kp == 0), stop=(kp == 2), perf_mode=DR,
                            )
                scr = scr_pool.tile([128, 512], bf, tag="scr")
                nc.vector.tensor_tensor(out=scr[:], in0=gm[:], in1=id4_bf[:], op=Alu.mult)
                nc.vector.tensor_reduce(
                    out=ssq[:, i0 : i0 + 4],
                    in_=scr.rearrange("p (q m) -> p q m", q=4),
                    axis=mybir.AxisListType.X, op=Alu.add,
                )

            for g in range(GROUPS_PER_CORE):
                # ============ phase B: fc chain ============
                fc_bf = tok_pool.tile([128, NT * C], bf, tag="fcbf", name=f"fcbf{g}")
                fcT = [tT_pool.tile([128, KC * 1024], bf, tag="fcT", name=f"fcT{g}_{hh}")
                       for hh in range(2)]
                ftT = [tT_pool.tile([128, KC * 1024], bf, tag="ftT", name=f"ftT{g}_{hh}")
                       for hh in range(2)]
                ssq_fc = stat_pool.tile([128, NT], dt.float32, tag="ssq_fc")
                ssq_ft = stat_pool.tile([128, NT], dt.float32, tag="ssq_ft")
                bprime = stat_pool.tile([128, NT], dt.float32, tag="bprime")
                B_sb = ab_pool.tile([128, M], bf, tag="Bmat")
                fcn = n8_pool.tile([128, KC * M], f8, tag="fcn", name=f"fcn{g}")
                # per-frame k-major transposes straight from DRAM:
                # T_h[p, k*1024+m] = x[frame h, tok m, c=k*128+p]
                for hh in range(2):
                    nc.sync.dma_start_transpose(
                        out=fcT[hh].rearrange("p (q m) -> p q m", m=1024),
                        in_=fc_d[2 * g + hh],
                    )
                    nc.sync.dma_start_transpose(
                        out=ftT[hh].rearrange("p (q m) -> p q m", m=1024),
                        in_=ft_d[2 * g + hh],
                    )
                nc.sync.dma_start(
                    out=fc_bf.rearrange("p (j c) -> p j c", j=NT),
                    in_=fc_d[2 * g : 2 * g + 2].rearrange("f (jj p) c -> p (f jj) c", p=128),
                )
                for i0 in range(0, NT, 4):
                    gram_bank(fcT, ssq_fc, i0, g, "cg")
                for i0 in range(0, NT, 4):
                    gram_bank(ftT, ssq_ft, i0, g, "fg")
                for h in range(2):
                    newton16(ssq_fc, bprime, slice(h * 8, (h + 1) * 8))
                    bcast_half(bprime, B_sb, h, g, "B")
                # fcn k-major: out col = k*2048 + n
                for h in range(2):
                    for k in range(KC):
                        eng = nc.vector if k % 2 == 0 else nc.gpsimd
                        eng.tensor_tensor(
                            out=fcn[:, k * M + h * 1024 : k * M + (h + 1) * 1024],
                            in0=fcT[h][:, k * 1024 : (k + 1) * 1024],
                            in1=B_sb[:, h * 1024 : (h + 1) * 1024],
                            op=Alu.mult,
                        )
                fcn3 = fcn.rearrange("p (k n) -> p k n", k=KC)

                ahat = stat_pool.tile([128, NT], dt.float32, tag="ahat")
                for h in range(2):
                    newton16(ssq_ft, ahat, slice(h * 8, (h + 1) * 8))
                A_sb = ab_pool.tile([128, M], bf, tag="Amat")
                for h in range(2):
                    bcast_half(ahat, A_sb, h, g, "A")
                # ftn in alt (j-major) layout: col = j*512 + k*128 + m
                ftn = n8_pool.tile([128, KC * M], f8, tag="ftn", name=f"ftn{g}")
                ftn3 = ftn.rearrange("p (k n) -> p k n", k=KC)
                for c in range(4):
                    for k in range(KC):
                        eng = nc.vector if k % 2 == 0 else nc.gpsimd
                        eng.tensor_tensor(
                            out=ftn[:, k * M + c * 512 : k * M + (c + 1) * 512],
                            in0=ftT[c // 2][:, k * 1024 + (c % 2) * 512 : k * 1024 + (c % 2) * 512 + 512],
                            in1=A_sb[:, c * 512 : (c + 1) * 512],
                            op=Alu.mult,
                        )

                # ============ main: logits GEMM (DR) + exp, n-half outer ============
                stot2 = stat_pool.tile([128, 2 * NT], dt.float32, tag="stot2")
                for i in range(NT):
                    for h in range(2):
                        lg = lg_psum.tile([128, 1024], dt.float32, tag="lg", name=f"lg{g}_{i}_{h}")
                        for nb in range(2):
                            blk = lg[:, nb * 512 : (nb + 1) * 512]
                            ncol = (2 * h + nb) * 512
                            for kp in (0, 2):
                                nc.tensor.matmul(
                                    blk,
                                    ftn3[:, kp : kp + 2, i * 128 : (i + 1) * 128],
                                    fcn3[:, kp : kp + 2, ncol : ncol + 512],
                                    start=(kp == 0), stop=(kp == 2), perf_mode=DR,
                                )
                        nc.scalar.activation(
                            lg[:], lg[:], Act.Exp, scale=POST,
                            accum_out=stot2[:, 2 * i + h : 2 * i + h + 1],
                        )

                # ============ phase D: diag cross-Grams, Q, pp/mavg ============
                # grp-mask work is independent; emitted here so its engine use
                # overlaps the ft/fc chains above.
                grp_sb = grp_pool.tile([S, M], dt.float32, tag="grp_sb")
                nc.sync.dma_start(
                    out=grp_sb.rearrange("s (f n) -> s f n", f=GF),
                    in_=gm_d[2 * g : 2 * g + 2].rearrange("f s n -> s f n"),
                )
                tpg = sm_psum.tile([128, NT * S], dt.float32, tag="sm", name=f"tpg{g}")
                for j in range(NT):
                    nc.tensor.transpose(
                        tpg[:, j * S : (j + 1) * S], grp_sb[:, j * 128 : (j + 1) * 128], id16_f32[:]
                    )
                grpT = grp_pool.tile([128, NT * S], dt.float32, tag="grpT")
                nc.vector.tensor_copy(grpT[:], tpg[:])
                rowmax = stat_pool.tile([128, NT], dt.float32, tag="rowmax")
                nc.vector.tensor_reduce(
                    out=rowmax[:],
                    in_=grpT.rearrange("p (j s) -> p j s", j=NT),
                    axis=mybir.AxisListType.X, op=Alu.max,
                )
                oh_f32 = grp_pool.tile([128, NT * S], dt.float32, tag="oh_f32")
                oh_w = grp_pool.tile([128, NT * S], dt.float32, tag="oh_w")
                ohsum = stat_pool.tile([128, S], dt.float32, tag="ohsum")
                cntb = stat_pool.tile([128, S], dt.float32, tag="cntb")
                for j in range(NT):
                    sl = slice(j * S, (j + 1) * S)
                    nc.gpsimd.tensor_scalar(
                        out=oh_f32[:, sl], in0=grpT[:, sl],
                        scalar1=rowmax[:, j : j + 1], scalar2=None, op0=Alu.is_equal,
                    )
                    if j == 0:
                        nc.gpsimd.tensor_copy(ohsum[:], oh_f32[:, sl])
                    else:
                        nc.gpsimd.tensor_tensor(out=ohsum[:], in0=ohsum[:], in1=oh_f32[:, sl], op=Alu.add)
                nc.gpsimd.partition_all_reduce(
                    out_ap=cntb[:], in_ap=ohsum[:], channels=128, reduce_op=bass_isa.ReduceOp.add,
                )
                nc.gpsimd.tensor_scalar(out=cntb[:], in0=cntb[:], scalar1=1.0, scalar2=None, op0=Alu.max)
                nc.vector.reciprocal(out=cntb[:], in_=cntb[:])
                for j in range(NT):
                    sl = slice(j * S, (j + 1) * S)
                    nc.gpsimd.tensor_tensor(out=oh_w[:, sl], in0=oh_f32[:, sl], in1=cntb[:], op=Alu.mult)

                # cross-Gram diag (ftn alt-layout lhsT, fcn k-major rhs)
                diagF = stat_pool.tile([128, NT], dt.float32, tag="diagF")
                for i0 in range(0, NT, 4):
                    gmx = sm_psum.tile([128, 512], dt.float32, tag="sm", name=f"xg{g}_{i0}")
                    for di in range(4):
                        i = i0 + di
                        for kp in (0, 2):
                            nc.tensor.matmul(
                                gmx[:, di * 128 : (di + 1) * 128],
                                ftn3[:, kp : kp + 2, i * 128 : (i + 1) * 128],
                                fcn3[:, kp : kp + 2, i * 128 : (i + 1) * 128],
                                start=(kp == 0), stop=(kp == 2), perf_mode=DR,
                            )
                    scr = scr_pool.tile([128, 512], bf, tag="scr")
                    nc.vector.tensor_tensor(out=scr[:], in0=gmx[:], in1=id4_bf[:], op=Alu.mult)
                    nc.vector.tensor_reduce(
                        out=diagF[:, i0 : i0 + 4],
                        in_=scr.rearrange("p (q m) -> p q m", q=4),
                        axis=mybir.AxisListType.X, op=Alu.add,
                    )

                oh_b = grp_pool.tile([128, NT * S], bf, tag="oh_b")
                for j in range(NT):
                    sl = slice(j * S, (j + 1) * S)
                    nc.gpsimd.tensor_scalar(
                        out=oh_b[:, sl], in0=oh_f32[:, sl],
                        scalar1=bprime[:, j : j + 1], scalar2=None, op0=Alu.mult,
                    )
                qq = sm_psum.tile([S, C], dt.float32, tag="sm", name=f"qq{g}")
                for j in range(NT):
                    nc.tensor.matmul(
                        qq[:], oh_b[:, j * S : (j + 1) * S], fc_bf[:, j * C : (j + 1) * C],
                        start=(j == 0), stop=(j == NT - 1),
                    )
                q_sb = q_pool.tile([S, C], bf, tag="q_sb")
                nc.vector.tensor_copy(q_sb[:], qq[:])
                qtp = sm_psum.tile([128, KC * S], bf, tag="sm", name=f"qtp{g}")
                for k in range(KC):
                    nc.tensor.transpose(
                        qtp[:, k * S : (k + 1) * S], q_sb[:, k * 128 : (k + 1) * 128], id16_bf[:]
                    )
                qt_sb = q_pool.tile([128, KC * S], bf, tag="qt_sb")
                nc.vector.tensor_copy(qt_sb[:], qtp[:])

                mavg = stat_pool.tile([128, NT], dt.float32, tag="mavg")
                for i0 in range(0, NT, 4):
                    ppt = sm_psum.tile([128, 4 * S], dt.float32, tag="sm", name=f"pp{g}_{i0}")
                    for di in range(4):
                        i = i0 + di
                        for k in range(KC):
                            nc.tensor.matmul(
                                ppt[:, di * S : (di + 1) * S],
                                ftn[:, k * M + i * 128 : k * M + (i + 1) * 128],
                                qt_sb[:, k * S : (k + 1) * S],
                                start=(k == 0), stop=(k == KC - 1),
                            )
                    pscr = scr_pool.tile([128, 4 * S], dt.float32, tag="pscr")
                    nc.vector.tensor_tensor(
                        out=pscr[:], in0=ppt[:], in1=oh_w[:, i0 * S : (i0 + 4) * S], op=Alu.mult
                    )
                    nc.vector.tensor_reduce(
                        out=mavg[:, i0 : i0 + 4],
                        in_=pscr.rearrange("p (q s) -> p q s", q=4),
                        axis=mybir.AxisListType.X, op=Alu.add,
                    )

                # ============ finals (lse ln'd host-side from raw stot2) ============
                tsum = stat_pool.tile([128, NT], dt.float32, tag="tsum")
                nc.gpsimd.tensor_tensor(out=tsum[:], in0=diagF[:], in1=mavg[:], op=Alu.add)
                nc.gpsimd.tensor_tensor(out=acc[:, 0:NT], in0=acc[:, 0:NT], in1=tsum[:], op=Alu.add)
                nc.gpsimd.tensor_copy(acc[:, 16 + 32 * g : 16 + 32 * (g + 1)], stot2[:])

            nc.sync.dma_start(out=out_d[:, :], in_=acc[:])

    nc.compile()
    return nc


def kernel(feat_trainable: np.ndarray, feat_criterion: np.ndarray, grp_masks: np.ndarray) -> np.ndarray:
    from concourse.bass_utils import run_bass_kernel_spmd

    if "nc" not in _CACHE:
        _CACHE["nc"] = _build()
    nc = _CACHE["nc"]

    import ml_dtypes
    ft = np.ascontiguousarray(
        np.asarray(feat_trainable, dtype=np.float32).reshape(B * T, N, C).astype(ml_dtypes.bfloat16))
    fc = np.ascontiguousarray(
        np.asarray(feat_criterion, dtype=np.float32).reshape(B * T, N, C).astype(ml_dtypes.bfloat16))
    gm = np.ascontiguousarray(np.asarray(grp_masks, dtype=np.float32).reshape(B * T, S, N))

    in_maps = []
    for c in range(N_CORES):
        fr = slice(c * FRAMES_PER_CORE, (c + 1) * FRAMES_PER_CORE)
        in_maps.append({
            "ft": np.ascontiguousarray(ft[fr]),
            "fc": np.ascontiguousarray(fc[fr]),
            "gm": np.ascontiguousarray(gm[fr]),
        })

    import time
    last_err = None
    for attempt in range(4):
        try:
            res = run_bass_kernel_spmd(nc, in_maps, list(range(N_CORES)))
            break
        except Exception as e:  # wedged-device recovery: wait and retry
            last_err = e
            time.sleep(20 + 25 * attempt)
    else:
        raise last_err
    total = np.float64(0.0)
    for c in range(N_CORES):
        o = np.asarray(res.results[c]["out"], dtype=np.float64)
        total += POST * o[:, :NT].sum()
        st = o[:, 16:].reshape(128, GROUPS_PER_CORE, NT, 2).sum(axis=-1)
        total -= 2.0 * np.log(st).sum()
    loss = SCALE * total / (G * M) / 2.0
    return np.asarray(loss, dtype=np.float32)


if __name__ == "__main__":
    nc = _build()
    print("build OK")


# revision 7
# speedup vs baseline: 1.9478x; 1.0130x over previous
"""DECConsLoss Trainium2 kernel v3: 8-core data-parallel over groups.

Reference (per group g of G=32, M=2048 tokens, C=512):
  ft_n, fc_n = l2norm(ft), l2norm(fc);  grp[m] = argmax_s masks
  logits = ft_n @ fc_n^T / 0.1;  lse[m] = logsumexp(logits[m,:])
  semi[m] = scale*(mean_{n: grp[n]==grp[m]} logits[m,n] - lse)
  pos[m]  = scale*(logits[m,m] - lse);  loss = mean(semi+pos)/2

v3 pipeline (all primitives hardware-validated):
  - SWDGE casting DMA loads fp32 DRAM -> bf16 SBUF token-major [128,(j,c)]
  - DMA XBAR transposes bf16 SBUF->SBUF -> [128,(j,k,m)] ("alt" layout)
  - ssq via plain bf16 Gram matmuls; diag extracted by tensor_tensor
    against a 4-wide identity + segmented tensor_reduce
  - 16*rsqrt(ssq) via Newton iteration on gpsimd (no ACT table thrash)
  - both tensors scaled to fp8 with broadcast matrices A (j-major out,
    gating per chunk) / B (k-major out, gating per n-half); exp scale is
    then the constant 10/256
  - main loop runs n-half-outer so the first 16 exps only need fc's
    first-half chain; ft's full chain is shorter
  - per-i diag from fp8 DR cross-Grams; masked means via Q/P side-GEMMs
  - exp in-place on PSUM with accum_out row-sums; ln(sums) on the host
Host: loss = SCALE*((10/256)*sum(acc0) - 2*sum(ln(stot))) / (G*M) / 2.
"""

import sys
import numpy as np

for p in ("/opt/trn_rl_repo", "/opt/trn_rl_repo/concourse", "/opt/pypackages"):
    if p not in sys.path:
        sys.path.insert(0, p)

GF = 2
S = 16
N = 1024
C = 512
B, T = 8, 8
G = (B * T) // GF            # 32 groups total
M = GF * N                   # 2048 tokens per group
N_CORES = 8
GROUPS_PER_CORE = G // N_CORES   # 4
FRAMES_PER_CORE = GROUPS_PER_CORE * GF  # 8
TEMP = 0.1
BASE_TEMP = 0.07
INV_TEMP = 1.0 / TEMP
SCALE = -(TEMP / BASE_TEMP)
FS = 16.0                    # fp8 pre-scale on both normalized tensors
POST = INV_TEMP / (FS * FS)  # 10/256: logits = POST * H

NT = M // 128                # 16 token tiles per group
KC = C // 128                # 4 contraction chunks

_CACHE = {}


def _build():
    import math
    import concourse.mybir as mybir
    from concourse import bacc
    from concourse import masks
    from concourse import bass_isa
    from concourse.tile import TileContext

    dt = mybir.dt
    Alu = mybir.AluOpType
    Act = mybir.ActivationFunctionType
    DR = mybir.MatmulPerfMode.DoubleRow

    nc = bacc.Bacc()
    ft_d = nc.declare_dram_parameter("ft", [FRAMES_PER_CORE, N, C], dt.bfloat16, isOutput=False)
    fc_d = nc.declare_dram_parameter("fc", [FRAMES_PER_CORE, N, C], dt.bfloat16, isOutput=False)
    gm_d = nc.declare_dram_parameter("gm", [FRAMES_PER_CORE, S, N], dt.float32, isOutput=False)
    out_d = nc.declare_dram_parameter("out", [128, 16 + 32 * GROUPS_PER_CORE], dt.float32, isOutput=True)

    f8 = dt.float8e4
    bf = dt.bfloat16

    with TileContext(nc) as tc:
        with (
            tc.tile_pool(name="consts", bufs=1) as consts,
            tc.tile_pool(name="tok_pool", bufs=2) as tok_pool,
            tc.tile_pool(name="tT_pool", bufs=4) as tT_pool,
            tc.tile_pool(name="n8_pool", bufs=2) as n8_pool,
            tc.tile_pool(name="ab_pool", bufs=1) as ab_pool,
            tc.tile_pool(name="grp_pool", bufs=2) as grp_pool,
            tc.tile_pool(name="stat_pool", bufs=2) as stat_pool,
            tc.tile_pool(name="scr_pool", bufs=2) as scr_pool,
            tc.tile_pool(name="q_pool", bufs=2) as q_pool,
            tc.tile_pool(name="acc_pool", bufs=1) as acc_pool,
            tc.tile_pool(name="sm_psum", bufs=2, space="PSUM") as sm_psum,
            tc.tile_pool(name="lg_psum", bufs=3, space="PSUM") as lg_psum,
        ):
            # ---- constants ----
            id128_bf = consts.tile([128, 128], bf)
            id128_f32 = consts.tile([128, 128], dt.float32)
            id16_bf = consts.tile([S, S], bf)
            id16_f32 = consts.tile([S, S], dt.float32)
            for t in (id128_bf, id128_f32, id16_bf, id16_f32):
                nc.vector.memset(t[:], 0.0)
                masks.make_identity(nc, t[:], nomemset=True)
            id4_bf = consts.tile([128, 512], bf)
            for q in range(4):
                nc.gpsimd.tensor_copy(id4_bf[:, q * 128 : (q + 1) * 128], id128_bf[:])
            # E8[:, jj*128:(jj+1)*128] = row-jj selector over 8 partitions
            E8 = consts.tile([8, 8 * 128], bf)
            nc.gpsimd.memset(E8[:], 0.0)
            nc.gpsimd.affine_select(
                out=E8[:], in_=E8[:],
                compare_op=mybir.AluOpType.not_equal, fill=1.0, base=0,
                pattern=[[-1, 8], [0, 128]], channel_multiplier=1,
            )

            acc = acc_pool.tile([128, 16 + 32 * GROUPS_PER_CORE], dt.float32)
            nc.vector.memset(acc[:, 0:16], 0.0)

            w0 = FS / math.sqrt(512.0)

            def newton16(ssq, dst, cols):
                """dst[:, cols] = FS*rsqrt(max(ssq,eps)) via 3 Newton steps."""
                sq, dv = ssq[:, cols], dst[:, cols]
                wscr = stat_pool.tile([128, 8], dt.float32, tag="wscr")
                nc.gpsimd.tensor_scalar(
                    out=sq, in0=sq, scalar1=1e-12, scalar2=1.0 / (FS * FS),
                    op0=Alu.max, op1=Alu.mult,
                )
                nc.gpsimd.tensor_scalar(
                    out=dv, in0=sq, scalar1=-0.5 * w0 ** 3, scalar2=1.5 * w0,
                    op0=Alu.mult, op1=Alu.add,
                )
                for _ in range(2):
                    nc.gpsimd.tensor_tensor(out=wscr[:], in0=dv, in1=dv, op=Alu.mult)
                    nc.gpsimd.tensor_tensor(out=wscr[:], in0=wscr[:], in1=sq, op=Alu.mult)
                    nc.gpsimd.tensor_scalar(
                        out=wscr[:], in0=wscr[:], scalar1=-0.5, scalar2=1.5,
                        op0=Alu.mult, op1=Alu.add,
                    )
                    nc.gpsimd.tensor_tensor(out=dv, in0=dv, in1=wscr[:], op=Alu.mult)

            def bcast_half(src, mat, h, g, nm):
                """mat[:, h*1024:(h+1)*1024] <- broadcast rows of src[:, h*8:(h+1)*8]."""
                tps = sm_psum.tile([8, 128], dt.float32, tag="sm", name=f"{nm}tp{g}_{h}")
                nc.tensor.transpose(tps[:], src[:, h * 8 : (h + 1) * 8], id128_f32[:])
                row_sb = q_pool.tile([8, 128], bf, tag=f"{nm}row")
                nc.vector.tensor_copy(row_sb[:], tps[:])
                for half_q in range(2):
                    bps = sm_psum.tile([128, 512], dt.float32, tag="sm", name=f"{nm}ps{g}_{h}_{half_q}")
                    for jj in range(4):
                        j = half_q * 4 + jj
                        nc.tensor.matmul(
                            bps[:, jj * 128 : (jj + 1) * 128],
                            E8[:, j * 128 : (j + 1) * 128], row_sb[:],
                            start=True, stop=True,
                        )
                    nc.vector.tensor_copy(
                        mat[:, h * 1024 + half_q * 512 : h * 1024 + (half_q + 1) * 512], bps[:]
                    )

            def gram_bank(T_bf, ssq, i0, g, nm, src3=None):
                gm = sm_psum.tile([128, 512], dt.float32, tag="sm", name=f"{nm}{g}_{i0}")
                for di in range(4):
                    i = i0 + di
                    if src3 is None:
                        for k in range(KC):
                            sl = slice(k * 1024 + (i % 8) * 128, k * 1024 + (i % 8 + 1) * 128)
                            nc.tensor.matmul(
                                gm[:, di * 128 : (di + 1) * 128],
                                T_bf[i // 8][:, sl], T_bf[i // 8][:, sl],
                                start=(k == 0), stop=(k == KC - 1),
                            )
                    else:
                        lhs3, rhs3 = src3
                        for kp in (0, 2):
                            nc.tensor.matmul(
                                gm[:, di * 128 : (di + 1) * 128],
                                lhs3[:, kp : kp + 2, i * 128 : (i + 1) * 128],
                                rhs3[:, kp : kp + 2, i * 128 : (i + 1) * 128],
                                start=(kp == 0), stop=(kp == 2), perf_mode=DR,
                            )
                scr = scr_pool.tile([128, 512], bf, tag="scr")
                nc.vector.tensor_tensor(out=scr[:], in0=gm[:], in1=id4_bf[:], op=Alu.mult)
                nc.vector.tensor_reduce(
                    out=ssq[:, i0 : i0 + 4],
                    in_=scr.rearrange("p (q m) -> p q m", q=4),
                    axis=mybir.AxisListType.X, op=Alu.add,
                )

            for g in range(GROUPS_PER_CORE):
                # ============ phase B: fc chain ============
                fc_bf = tok_pool.tile([128, NT * C], bf, tag="fcbf", name=f"fcbf{g}")
                fcT = [tT_pool.tile([128, KC * 1024], bf, tag="fcT", name=f"fcT{g}_{hh}")
                       for hh in range(2)]
                ftT = [tT_pool.tile([128, KC * 1024], bf, tag="ftT", name=f"ftT{g}_{hh}")
                       for hh in range(2)]
                ssq_fc = stat_pool.tile([128, NT], dt.float32, tag="ssq_fc")
                ssq_ft = stat_pool.tile([128, NT], dt.float32, tag="ssq_ft")
                bprime = stat_pool.tile([128, NT], dt.float32, tag="bprime")
                B_sb = ab_pool.tile([128, M], bf, tag="Bmat")
                fcn = n8_pool.tile([128, KC * M], f8, tag="fcn", name=f"fcn{g}")
                # per-frame k-major transposes straight from DRAM:
                # T_h[p, k*1024+m] = x[frame h, tok m, c=k*128+p]
                for hh in range(2):
                    nc.sync.dma_start_transpose(
                        out=fcT[hh].rearrange("p (q m) -> p q m", m=1024),
                        in_=fc_d[2 * g + hh],
                    )
                    nc.sync.dma_start_transpose(
                        out=ftT[hh].rearrange("p (q m) -> p q m", m=1024),
                        in_=ft_d[2 * g + hh],
                    )
                nc.sync.dma_start(
                    out=fc_bf.rearrange("p (j c) -> p j c", j=NT),
                    in_=fc_d[2 * g : 2 * g + 2].rearrange("f (jj p) c -> p (f jj) c", p=128),
                )
                for i0 in range(0, NT, 4):
                    gram_bank(fcT, ssq_fc, i0, g, "cg")
                for i0 in range(0, NT, 4):
                    gram_bank(ftT, ssq_ft, i0, g, "fg")
                for h in range(2):
                    newton16(ssq_fc, bprime, slice(h * 8, (h + 1) * 8))
                    bcast_half(bprime, B_sb, h, g, "B")
                # fcn k-major: out col = k*2048 + n
                for h in range(2):
                    for k in range(KC):
                        eng = nc.vector if k % 2 == 0 else nc.gpsimd
                        eng.tensor_tensor(
                            out=fcn[:, k * M + h * 1024 : k * M + (h + 1) * 1024],
                            in0=fcT[h][:, k * 1024 : (k + 1) * 1024],
                            in1=B_sb[:, h * 1024 : (h + 1) * 1024],
                            op=Alu.mult,
                        )
                fcn3 = fcn.rearrange("p (k n) -> p k n", k=KC)

                ahat = stat_pool.tile([128, NT], dt.float32, tag="ahat")
                for h in range(2):
                    newton16(ssq_ft, ahat, slice(h * 8, (h + 1) * 8))
                A_sb = ab_pool.tile([128, M], bf, tag="Amat")
                for h in range(2):
                    bcast_half(ahat, A_sb, h, g, "A")
                # ftn in alt (j-major) layout: col = j*512 + k*128 + m
                ftn = n8_pool.tile([128, KC * M], f8, tag="ftn", name=f"ftn{g}")
                ftn3 = ftn.rearrange("p (k n) -> p k n", k=KC)
                for c in range(4):
                    for k in range(KC):
                        eng = nc.vector if k % 2 == 0 else nc.gpsimd
                        eng.tensor_tensor(
                            out=ftn[:, k * M + c * 512 : k * M + (c + 1) * 512],
                            in0=ftT[c // 2][:, k * 1024 + (c % 2) * 512 : k * 1024 + (c % 2) * 512 + 512],
                            in1=A_sb[:, c * 512 : (c + 1) * 512],
                            op=Alu.mult,
                        )

                def phase_d(g, fc_bf, ftn, fcn3, bprime, stot2):
                    ftn3 = ftn.rearrange("p (k n) -> p k n", k=KC)

                    # grp-mask work is independent; emitted here so its engine use
                    # overlaps the ft/fc chains above.
                    grp_sb = grp_pool.tile([S, M], dt.float32, tag="grp_sb")
                    nc.sync.dma_start(
                        out=grp_sb.rearrange("s (f n) -> s f n", f=GF),
                        in_=gm_d[2 * g : 2 * g + 2].rearrange("f s n -> s f n"),
                    )
                    tpg = sm_psum.tile([128, NT * S], dt.float32, tag="sm", name=f"tpg{g}")
                    for j in range(NT):
                        nc.tensor.transpose(
                            tpg[:, j * S : (j + 1) * S], grp_sb[:, j * 128 : (j + 1) * 128], id16_f32[:]
                        )
                    grpT = grp_pool.tile([128, NT * S], dt.float32, tag="grpT")
                    nc.vector.tensor_copy(grpT[:], tpg[:])
                    rowmax = stat_pool.tile([128, NT], dt.float32, tag="rowmax")
                    nc.vector.tensor_reduce(
                        out=rowmax[:],
                        in_=grpT.rearrange("p (j s) -> p j s", j=NT),
                        axis=mybir.AxisListType.X, op=Alu.max,
                    )
                    oh_f32 = grp_pool.tile([128, NT * S], dt.float32, tag="oh_f32")
                    oh_w = grp_pool.tile([128, NT * S], dt.float32, tag="oh_w")
                    ohsum = stat_pool.tile([128, S], dt.float32, tag="ohsum")
                    cntb = stat_pool.tile([128, S], dt.float32, tag="cntb")
                    for j in range(NT):
                        sl = slice(j * S, (j + 1) * S)
                        nc.gpsimd.tensor_scalar(
                            out=oh_f32[:, sl], in0=grpT[:, sl],
                            scalar1=rowmax[:, j : j + 1], scalar2=None, op0=Alu.is_equal,
                        )
                        if j == 0:
                            nc.gpsimd.tensor_copy(ohsum[:], oh_f32[:, sl])
                        else:
                            nc.gpsimd.tensor_tensor(out=ohsum[:], in0=ohsum[:], in1=oh_f32[:, sl], op=Alu.add)
                    nc.gpsimd.partition_all_reduce(
                        out_ap=cntb[:], in_ap=ohsum[:], channels=128, reduce_op=bass_isa.ReduceOp.add,
                    )
                    nc.gpsimd.tensor_scalar(out=cntb[:], in0=cntb[:], scalar1=1.0, scalar2=None, op0=Alu.max)
                    nc.vector.reciprocal(out=cntb[:], in_=cntb[:])
                    for j in range(NT):
                        sl = slice(j * S, (j + 1) * S)
                        nc.gpsimd.tensor_tensor(out=oh_w[:, sl], in0=oh_f32[:, sl], in1=cntb[:], op=Alu.mult)

                    # cross-Gram diag (ftn alt-layout lhsT, fcn k-major rhs)
                    diagF = stat_pool.tile([128, NT], dt.float32, tag="diagF")
                    for i0 in range(0, NT, 4):
                        gmx = sm_psum.tile([128, 512], dt.float32, tag="sm", name=f"xg{g}_{i0}")
                        for di in range(4):
                            i = i0 + di
                            for kp in (0, 2):
                                nc.tensor.matmul(
                                    gmx[:, di * 128 : (di + 1) * 128],
                                    ftn3[:, kp : kp + 2, i * 128 : (i + 1) * 128],
                                    fcn3[:, kp : kp + 2, i * 128 : (i + 1) * 128],
                                    start=(kp == 0), stop=(kp == 2), perf_mode=DR,
                                )
                        scr = scr_pool.tile([128, 512], bf, tag="scr")
                        nc.vector.tensor_tensor(out=scr[:], in0=gmx[:], in1=id4_bf[:], op=Alu.mult)
                        nc.vector.tensor_reduce(
                            out=diagF[:, i0 : i0 + 4],
                            in_=scr.rearrange("p (q m) -> p q m", q=4),
                            axis=mybir.AxisListType.X, op=Alu.add,
                        )

                    oh_b = grp_pool.tile([128, NT * S], bf, tag="oh_b")
                    for j in range(NT):
                        sl = slice(j * S, (j + 1) * S)
                        nc.gpsimd.tensor_scalar(
                            out=oh_b[:, sl], in0=oh_f32[:, sl],
                            scalar1=bprime[:, j : j + 1], scalar2=None, op0=Alu.mult,
                        )
                    qq = sm_psum.tile([S, C], dt.float32, tag="sm", name=f"qq{g}")
                    for j in range(NT):
                        nc.tensor.matmul(
                            qq[:], oh_b[:, j * S : (j + 1) * S], fc_bf[:, j * C : (j + 1) * C],
                            start=(j == 0), stop=(j == NT - 1),
                        )
                    q_sb = q_pool.tile([S, C], bf, tag="q_sb")
                    nc.vector.tensor_copy(q_sb[:], qq[:])
                    qtp = sm_psum.tile([128, KC * S], bf, tag="sm", name=f"qtp{g}")
                    for k in range(KC):
                        nc.tensor.transpose(
                            qtp[:, k * S : (k + 1) * S], q_sb[:, k * 128 : (k + 1) * 128], id16_bf[:]
                        )
                    qt_sb = q_pool.tile([128, KC * S], bf, tag="qt_sb")
                    nc.vector.tensor_copy(qt_sb[:], qtp[:])

                    mavg = stat_pool.tile([128, NT], dt.float32, tag="mavg")
                    for i0 in range(0, NT, 4):
                        ppt = sm_psum.tile([128, 4 * S], dt.float32, tag="sm", name=f"pp{g}_{i0}")
                        for di in range(4):
                            i = i0 + di
                            for k in range(KC):
                                nc.tensor.matmul(
                                    ppt[:, di * S : (di + 1) * S],
                                    ftn[:, k * M + i * 128 : k * M + (i + 1) * 128],
                                    qt_sb[:, k * S : (k + 1) * S],
                                    start=(k == 0), stop=(k == KC - 1),
                                )
                        pscr = scr_pool.tile([128, 4 * S], dt.float32, tag="pscr")
                        nc.vector.tensor_tensor(
                            out=pscr[:], in0=ppt[:], in1=oh_w[:, i0 * S : (i0 + 4) * S], op=Alu.mult
                        )
                        nc.vector.tensor_reduce(
                            out=mavg[:, i0 : i0 + 4],
                            in_=pscr.rearrange("p (q s) -> p q s", q=4),
                            axis=mybir.AxisListType.X, op=Alu.add,
                        )

                    # ============ finals (lse ln'd host-side from raw stot2) ============
                    tsum = stat_pool.tile([128, NT], dt.float32, tag="tsum")
                    nc.gpsimd.tensor_tensor(out=tsum[:], in0=diagF[:], in1=mavg[:], op=Alu.add)
                    nc.gpsimd.tensor_tensor(out=acc[:, 0:NT], in0=acc[:, 0:NT], in1=tsum[:], op=Alu.add)
                    nc.gpsimd.tensor_copy(acc[:, 16 + 32 * g : 16 + 32 * (g + 1)], stot2[:])


                stot2 = stat_pool.tile([128, 2 * NT], dt.float32, tag="stot2")
                if g > 0:
                    phase_d(g - 1, *deferred)
                deferred = (fc_bf, ftn, fcn3, bprime, stot2)

                # ============ main: logits GEMM (DR) + exp, n-half outer ============
                for i in range(NT):
                    for h in range(2):
                        lg = lg_psum.tile([128, 1024], dt.float32, tag="lg", name=f"lg{g}_{i}_{h}")
                        for nb in range(2):
                            blk = lg[:, nb * 512 : (nb + 1) * 512]
                            ncol = (2 * h + nb) * 512
                            for kp in (0, 2):
                                nc.tensor.matmul(
                                    blk,
                                    ftn3[:, kp : kp + 2, i * 128 : (i + 1) * 128],
                                    fcn3[:, kp : kp + 2, ncol : ncol + 512],
                                    start=(kp == 0), stop=(kp == 2), perf_mode=DR,
                                )
                        nc.scalar.activation(
                            lg[:], lg[:], Act.Exp, scale=POST,
                            accum_out=stot2[:, 2 * i + h : 2 * i + h + 1],
                        )

            phase_d(GROUPS_PER_CORE - 1, *deferred)
            nc.sync.dma_start(out=out_d[:, :], in_=acc[:])

    nc.compile()
    return nc


def kernel(feat_trainable: np.ndarray, feat_criterion: np.ndarray, grp_masks: np.ndarray) -> np.ndarray:
    from concourse.bass_utils import run_bass_kernel_spmd

    if "nc" not in _CACHE:
        _CACHE["nc"] = _build()
    nc = _CACHE["nc"]

    import ml_dtypes
    ft = np.ascontiguousarray(
        np.asarray(feat_trainable, dtype=np.float32).reshape(B * T, N, C).astype(ml_dtypes.bfloat16))
    fc = np.ascontiguousarray(
        np.asarray(feat_criterion, dtype=np.float32).reshape(B * T, N, C).astype(ml_dtypes.bfloat16))
    gm = np.ascontiguousarray(np.asarray(grp_masks, dtype=np.float32).reshape(B * T, S, N))

    in_maps = []
    for c in range(N_CORES):
        fr = slice(c * FRAMES_PER_CORE, (c + 1) * FRAMES_PER_CORE)
        in_maps.append({
            "ft": np.ascontiguousarray(ft[fr]),
            "fc": np.ascontiguousarray(fc[fr]),
            "gm": np.ascontiguousarray(gm[fr]),
        })

    import time
    last_err = None
    for attempt in range(4):
        try:
            res = run_bass_kernel_spmd(nc, in_maps, list(range(N_CORES)))
            break
        except Exception as e:  # wedged-device recovery: wait and retry
            last_err = e
            time.sleep(20 + 25 * attempt)
    else:
        raise last_err
    total = np.float64(0.0)
    for c in range(N_CORES):
        o = np.asarray(res.results[c]["out"], dtype=np.float64)
        total += POST * o[:, :NT].sum()
        st = o[:, 16:].reshape(128, GROUPS_PER_CORE, NT, 2).sum(axis=-1)
        total -= 2.0 * np.log(st).sum()
    loss = SCALE * total / (G * M) / 2.0
    return np.asarray(loss, dtype=np.float32)


if __name__ == "__main__":
    nc = _build()
    print("build OK")


# revision 8
# speedup vs baseline: 1.9675x; 1.0101x over previous
"""DECConsLoss Trainium2 kernel v3: 8-core data-parallel over groups.

Reference (per group g of G=32, M=2048 tokens, C=512):
  ft_n, fc_n = l2norm(ft), l2norm(fc);  grp[m] = argmax_s masks
  logits = ft_n @ fc_n^T / 0.1;  lse[m] = logsumexp(logits[m,:])
  semi[m] = scale*(mean_{n: grp[n]==grp[m]} logits[m,n] - lse)
  pos[m]  = scale*(logits[m,m] - lse);  loss = mean(semi+pos)/2

v3 pipeline (all primitives hardware-validated):
  - SWDGE casting DMA loads fp32 DRAM -> bf16 SBUF token-major [128,(j,c)]
  - DMA XBAR transposes bf16 SBUF->SBUF -> [128,(j,k,m)] ("alt" layout)
  - ssq via plain bf16 Gram matmuls; diag extracted by tensor_tensor
    against a 4-wide identity + segmented tensor_reduce
  - 16*rsqrt(ssq) via Newton iteration on gpsimd (no ACT table thrash)
  - both tensors scaled to fp8 with broadcast matrices A (j-major out,
    gating per chunk) / B (k-major out, gating per n-half); exp scale is
    then the constant 10/256
  - main loop runs n-half-outer so the first 16 exps only need fc's
    first-half chain; ft's full chain is shorter
  - per-i diag from fp8 DR cross-Grams; masked means via Q/P side-GEMMs
  - exp in-place on PSUM with accum_out row-sums; ln(sums) on the host
Host: loss = SCALE*((10/256)*sum(acc0) - 2*sum(ln(stot))) / (G*M) / 2.
"""

import sys
import numpy as np

for p in ("/opt/trn_rl_repo", "/opt/trn_rl_repo/concourse", "/opt/pypackages"):
    if p not in sys.path:
        sys.path.insert(0, p)

GF = 2
S = 16
N = 1024
C = 512
B, T = 8, 8
G = (B * T) // GF            # 32 groups total
M = GF * N                   # 2048 tokens per group
N_CORES = 8
GROUPS_PER_CORE = G // N_CORES   # 4
FRAMES_PER_CORE = GROUPS_PER_CORE * GF  # 8
TEMP = 0.1
BASE_TEMP = 0.07
INV_TEMP = 1.0 / TEMP
SCALE = -(TEMP / BASE_TEMP)
FS = 16.0                    # fp8 pre-scale on both normalized tensors
POST = INV_TEMP / (FS * FS)  # 10/256: logits = POST * H

NT = M // 128                # 16 token tiles per group
KC = C // 128                # 4 contraction chunks

_CACHE = {}


def _build():
    import math
    import concourse.mybir as mybir
    from concourse import bacc
    from concourse import masks
    from concourse import bass_isa
    from concourse.tile import TileContext

    dt = mybir.dt
    Alu = mybir.AluOpType
    Act = mybir.ActivationFunctionType
    DR = mybir.MatmulPerfMode.DoubleRow

    nc = bacc.Bacc()
    ft_d = nc.declare_dram_parameter("ft", [FRAMES_PER_CORE, N, C], dt.bfloat16, isOutput=False)
    fc_d = nc.declare_dram_parameter("fc", [FRAMES_PER_CORE, N, C], dt.bfloat16, isOutput=False)
    gm_d = nc.declare_dram_parameter("gm", [FRAMES_PER_CORE, S, N], dt.float32, isOutput=False)
    out_d = nc.declare_dram_parameter("out", [128, 16 + 32 * GROUPS_PER_CORE], dt.float32, isOutput=True)

    f8 = dt.float8e4
    bf = dt.bfloat16

    with TileContext(nc) as tc:
        with (
            tc.tile_pool(name="consts", bufs=1) as consts,
            tc.tile_pool(name="tok_pool", bufs=2) as tok_pool,
            tc.tile_pool(name="tT_pool", bufs=4) as tT_pool,
            tc.tile_pool(name="n8_pool", bufs=2) as n8_pool,
            tc.tile_pool(name="ab_pool", bufs=1) as ab_pool,
            tc.tile_pool(name="grp_pool", bufs=2) as grp_pool,
            tc.tile_pool(name="stat_pool", bufs=2) as stat_pool,
            tc.tile_pool(name="scr_pool", bufs=2) as scr_pool,
            tc.tile_pool(name="q_pool", bufs=2) as q_pool,
            tc.tile_pool(name="acc_pool", bufs=1) as acc_pool,
            tc.tile_pool(name="sm_psum", bufs=2, space="PSUM") as sm_psum,
            tc.tile_pool(name="lg_psum", bufs=3, space="PSUM") as lg_psum,
        ):
            # ---- constants ----
            id128_bf = consts.tile([128, 128], bf)
            id128_f32 = consts.tile([128, 128], dt.float32)
            id16_bf = consts.tile([S, S], bf)
            id16_f32 = consts.tile([S, S], dt.float32)
            for t in (id128_bf, id128_f32, id16_bf, id16_f32):
                nc.vector.memset(t[:], 0.0)
                masks.make_identity(nc, t[:], nomemset=True)
            id4_bf = consts.tile([128, 512], bf)
            for q in range(4):
                nc.gpsimd.tensor_copy(id4_bf[:, q * 128 : (q + 1) * 128], id128_bf[:])
            # E8[:, jj*128:(jj+1)*128] = row-jj selector over 8 partitions
            E8 = consts.tile([8, 8 * 128], bf)
            nc.gpsimd.memset(E8[:], 0.0)
            nc.gpsimd.affine_select(
                out=E8[:], in_=E8[:],
                compare_op=mybir.AluOpType.not_equal, fill=1.0, base=0,
                pattern=[[-1, 8], [0, 128]], channel_multiplier=1,
            )

            acc = acc_pool.tile([128, 16 + 32 * GROUPS_PER_CORE], dt.float32)
            nc.vector.memset(acc[:, 0:16], 0.0)

            w0 = FS / math.sqrt(512.0)

            def newton16(ssq, dst, cols):
                """dst[:, cols] = FS*rsqrt(max(ssq,eps)) via 3 Newton steps."""
                sq, dv = ssq[:, cols], dst[:, cols]
                wscr = stat_pool.tile([128, 8], dt.float32, tag="wscr")
                nc.gpsimd.tensor_scalar(
                    out=sq, in0=sq, scalar1=1e-12, scalar2=1.0 / (FS * FS),
                    op0=Alu.max, op1=Alu.mult,
                )
                nc.gpsimd.tensor_scalar(
                    out=dv, in0=sq, scalar1=-0.5 * w0 ** 3, scalar2=1.5 * w0,
                    op0=Alu.mult, op1=Alu.add,
                )
                for _ in range(2):
                    nc.gpsimd.tensor_tensor(out=wscr[:], in0=dv, in1=dv, op=Alu.mult)
                    nc.gpsimd.tensor_tensor(out=wscr[:], in0=wscr[:], in1=sq, op=Alu.mult)
                    nc.gpsimd.tensor_scalar(
                        out=wscr[:], in0=wscr[:], scalar1=-0.5, scalar2=1.5,
                        op0=Alu.mult, op1=Alu.add,
                    )
                    nc.gpsimd.tensor_tensor(out=dv, in0=dv, in1=wscr[:], op=Alu.mult)

            def bcast_half(src, mat, h, g, nm):
                """mat[:, h*1024:(h+1)*1024] <- broadcast rows of src[:, h*8:(h+1)*8]."""
                tps = sm_psum.tile([8, 128], dt.float32, tag="sm", name=f"{nm}tp{g}_{h}")
                nc.tensor.transpose(tps[:], src[:, h * 8 : (h + 1) * 8], id128_f32[:])
                row_sb = q_pool.tile([8, 128], bf, tag=f"{nm}row")
                nc.vector.tensor_copy(row_sb[:], tps[:])
                for half_q in range(2):
                    bps = sm_psum.tile([128, 512], dt.float32, tag="sm", name=f"{nm}ps{g}_{h}_{half_q}")
                    for jj in range(4):
                        j = half_q * 4 + jj
                        nc.tensor.matmul(
                            bps[:, jj * 128 : (jj + 1) * 128],
                            E8[:, j * 128 : (j + 1) * 128], row_sb[:],
                            start=True, stop=True,
                        )
                    nc.vector.tensor_copy(
                        mat[:, h * 1024 + half_q * 512 : h * 1024 + (half_q + 1) * 512], bps[:]
                    )

            def gram_bank(T_bf, ssq, i0, g, nm, src3=None):
                gm = sm_psum.tile([128, 512], dt.float32, tag="sm", name=f"{nm}{g}_{i0}")
                for di in range(4):
                    i = i0 + di
                    if src3 is None:
                        for k in range(KC):
                            sl = slice(k * 1024 + (i % 8) * 128, k * 1024 + (i % 8 + 1) * 128)
                            nc.tensor.matmul(
                                gm[:, di * 128 : (di + 1) * 128],
                                T_bf[i // 8][:, sl], T_bf[i // 8][:, sl],
                                start=(k == 0), stop=(k == KC - 1),
                            )
                    else:
                        lhs3, rhs3 = src3
                        for kp in (0, 2):
                            nc.tensor.matmul(
                                gm[:, di * 128 : (di + 1) * 128],
                                lhs3[:, kp : kp + 2, i * 128 : (i + 1) * 128],
                                rhs3[:, kp : kp + 2, i * 128 : (i + 1) * 128],
                                start=(kp == 0), stop=(kp == 2), perf_mode=DR,
                            )
                scr = scr_pool.tile([128, 512], bf, tag="scr")
                nc.vector.tensor_tensor(out=scr[:], in0=gm[:], in1=id4_bf[:], op=Alu.mult)
                nc.vector.tensor_reduce(
                    out=ssq[:, i0 : i0 + 4],
                    in_=scr.rearrange("p (q m) -> p q m", q=4),
                    axis=mybir.AxisListType.X, op=Alu.add,
                )

            for g in range(GROUPS_PER_CORE):
                # ============ phase B: fc chain ============
                fc_bf = tok_pool.tile([128, NT * C], bf, tag="fcbf", name=f"fcbf{g}")
                fcT = [tT_pool.tile([128, KC * 1024], bf, tag="fcT", name=f"fcT{g}_{hh}")
                       for hh in range(2)]
                ftT = [tT_pool.tile([128, KC * 1024], bf, tag="ftT", name=f"ftT{g}_{hh}")
                       for hh in range(2)]
                ssq_fc = stat_pool.tile([128, NT], dt.float32, tag="ssq_fc")
                ssq_ft = stat_pool.tile([128, NT], dt.float32, tag="ssq_ft")
                bprime = stat_pool.tile([128, NT], dt.float32, tag="bprime")
                B_sb = ab_pool.tile([128, M], bf, tag="Bmat")
                fcn = n8_pool.tile([128, KC * M], f8, tag="fcn", name=f"fcn{g}")
                # per-frame k-major transposes straight from DRAM:
                # T_h[p, k*1024+m] = x[frame h, tok m, c=k*128+p]
                for hh in range(2):
                    nc.sync.dma_start_transpose(
                        out=fcT[hh].rearrange("p (q m) -> p q m", m=1024),
                        in_=fc_d[2 * g + hh],
                    )
                    nc.sync.dma_start_transpose(
                        out=ftT[hh].rearrange("p (q m) -> p q m", m=1024),
                        in_=ft_d[2 * g + hh],
                    )
                nc.sync.dma_start(
                    out=fc_bf.rearrange("p (j c) -> p j c", j=NT),
                    in_=fc_d[2 * g : 2 * g + 2].rearrange("f (jj p) c -> p (f jj) c", p=128),
                )
                # interleave gram banks to match transpose completion order:
                # fcT-h0, ftT-h0, fcT-h1, ftT-h1
                for hh in range(2):
                    for i0 in (hh * 8, hh * 8 + 4):
                        gram_bank(fcT, ssq_fc, i0, g, "cg")
                    for i0 in (hh * 8, hh * 8 + 4):
                        gram_bank(ftT, ssq_ft, i0, g, "fg")
                for h in range(2):
                    newton16(ssq_fc, bprime, slice(h * 8, (h + 1) * 8))
                    bcast_half(bprime, B_sb, h, g, "B")
                # fcn k-major: out col = k*2048 + n
                for h in range(2):
                    for k in range(KC):
                        eng = nc.vector if k % 2 == 0 else nc.gpsimd
                        eng.tensor_tensor(
                            out=fcn[:, k * M + h * 1024 : k * M + (h + 1) * 1024],
                            in0=fcT[h][:, k * 1024 : (k + 1) * 1024],
                            in1=B_sb[:, h * 1024 : (h + 1) * 1024],
                            op=Alu.mult,
                        )
                fcn3 = fcn.rearrange("p (k n) -> p k n", k=KC)

                ahat = stat_pool.tile([128, NT], dt.float32, tag="ahat")
                for h in range(2):
                    newton16(ssq_ft, ahat, slice(h * 8, (h + 1) * 8))
                A_sb = ab_pool.tile([128, M], bf, tag="Amat")
                for h in range(2):
                    bcast_half(ahat, A_sb, h, g, "A")
                # ftn in alt (j-major) layout: col = j*512 + k*128 + m
                ftn = n8_pool.tile([128, KC * M], f8, tag="ftn", name=f"ftn{g}")
                ftn3 = ftn.rearrange("p (k n) -> p k n", k=KC)
                for c in range(4):
                    for k in range(KC):
                        eng = nc.vector if k % 2 == 0 else nc.gpsimd
                        eng.tensor_tensor(
                            out=ftn[:, k * M + c * 512 : k * M + (c + 1) * 512],
                            in0=ftT[c // 2][:, k * 1024 + (c % 2) * 512 : k * 1024 + (c % 2) * 512 + 512],
                            in1=A_sb[:, c * 512 : (c + 1) * 512],
                            op=Alu.mult,
                        )

                def phase_d(g, fc_bf, ftn, fcn3, bprime, stot2):
                    ftn3 = ftn.rearrange("p (k n) -> p k n", k=KC)

                    # grp-mask work is independent; emitted here so its engine use
                    # overlaps the ft/fc chains above.
                    grp_sb = grp_pool.tile([S, M], dt.float32, tag="grp_sb")
                    nc.sync.dma_start(
                        out=grp_sb.rearrange("s (f n) -> s f n", f=GF),
                        in_=gm_d[2 * g : 2 * g + 2].rearrange("f s n -> s f n"),
                    )
                    tpg = sm_psum.tile([128, NT * S], dt.float32, tag="sm", name=f"tpg{g}")
                    for j in range(NT):
                        nc.tensor.transpose(
                            tpg[:, j * S : (j + 1) * S], grp_sb[:, j * 128 : (j + 1) * 128], id16_f32[:]
                        )
                    grpT = grp_pool.tile([128, NT * S], dt.float32, tag="grpT")
                    nc.vector.tensor_copy(grpT[:], tpg[:])
                    rowmax = stat_pool.tile([128, NT], dt.float32, tag="rowmax")
                    nc.vector.tensor_reduce(
                        out=rowmax[:],
                        in_=grpT.rearrange("p (j s) -> p j s", j=NT),
                        axis=mybir.AxisListType.X, op=Alu.max,
                    )
                    oh_f32 = grp_pool.tile([128, NT * S], dt.float32, tag="oh_f32")
                    oh_w = grp_pool.tile([128, NT * S], dt.float32, tag="oh_w")
                    ohsum = stat_pool.tile([128, S], dt.float32, tag="ohsum")
                    cntb = stat_pool.tile([128, S], dt.float32, tag="cntb")
                    for j in range(NT):
                        sl = slice(j * S, (j + 1) * S)
                        nc.gpsimd.tensor_scalar(
                            out=oh_f32[:, sl], in0=grpT[:, sl],
                            scalar1=rowmax[:, j : j + 1], scalar2=None, op0=Alu.is_equal,
                        )
                        if j == 0:
                            nc.gpsimd.tensor_copy(ohsum[:], oh_f32[:, sl])
                        else:
                            nc.gpsimd.tensor_tensor(out=ohsum[:], in0=ohsum[:], in1=oh_f32[:, sl], op=Alu.add)
                    nc.gpsimd.partition_all_reduce(
                        out_ap=cntb[:], in_ap=ohsum[:], channels=128, reduce_op=bass_isa.ReduceOp.add,
                    )
                    nc.gpsimd.tensor_scalar(out=cntb[:], in0=cntb[:], scalar1=1.0, scalar2=None, op0=Alu.max)
                    nc.vector.reciprocal(out=cntb[:], in_=cntb[:])
                    for j in range(NT):
                        sl = slice(j * S, (j + 1) * S)
                        nc.gpsimd.tensor_tensor(out=oh_w[:, sl], in0=oh_f32[:, sl], in1=cntb[:], op=Alu.mult)

                    # cross-Gram diag (ftn alt-layout lhsT, fcn k-major rhs)
                    diagF = stat_pool.tile([128, NT], dt.float32, tag="diagF")
                    for i0 in range(0, NT, 4):
                        gmx = sm_psum.tile([128, 512], dt.float32, tag="sm", name=f"xg{g}_{i0}")
                        for di in range(4):
                            i = i0 + di
                            for kp in (0, 2):
                                nc.tensor.matmul(
                                    gmx[:, di * 128 : (di + 1) * 128],
                                    ftn3[:, kp : kp + 2, i * 128 : (i + 1) * 128],
                                    fcn3[:, kp : kp + 2, i * 128 : (i + 1) * 128],
                                    start=(kp == 0), stop=(kp == 2), perf_mode=DR,
                                )
                        scr = scr_pool.tile([128, 512], bf, tag="scr")
                        nc.vector.tensor_tensor(out=scr[:], in0=gmx[:], in1=id4_bf[:], op=Alu.mult)
                        nc.vector.tensor_reduce(
                            out=diagF[:, i0 : i0 + 4],
                            in_=scr.rearrange("p (q m) -> p q m", q=4),
                            axis=mybir.AxisListType.X, op=Alu.add,
                        )

                    oh_b = grp_pool.tile([128, NT * S], bf, tag="oh_b")
                    for j in range(NT):
                        sl = slice(j * S, (j + 1) * S)
                        nc.gpsimd.tensor_scalar(
                            out=oh_b[:, sl], in0=oh_f32[:, sl],
                            scalar1=bprime[:, j : j + 1], scalar2=None, op0=Alu.mult,
                        )
                    qq = sm_psum.tile([S, C], dt.float32, tag="sm", name=f"qq{g}")
                    for j in range(NT):
                        nc.tensor.matmul(
                            qq[:], oh_b[:, j * S : (j + 1) * S], fc_bf[:, j * C : (j + 1) * C],
                            start=(j == 0), stop=(j == NT - 1),
                        )
                    q_sb = q_pool.tile([S, C], bf, tag="q_sb")
                    nc.vector.tensor_copy(q_sb[:], qq[:])
                    qtp = sm_psum.tile([128, KC * S], bf, tag="sm", name=f"qtp{g}")
                    for k in range(KC):
                        nc.tensor.transpose(
                            qtp[:, k * S : (k + 1) * S], q_sb[:, k * 128 : (k + 1) * 128], id16_bf[:]
                        )
                    qt_sb = q_pool.tile([128, KC * S], bf, tag="qt_sb")
                    nc.vector.tensor_copy(qt_sb[:], qtp[:])

                    mavg = stat_pool.tile([128, NT], dt.float32, tag="mavg")
                    for i0 in range(0, NT, 4):
                        ppt = sm_psum.tile([128, 4 * S], dt.float32, tag="sm", name=f"pp{g}_{i0}")
                        for di in range(4):
                            i = i0 + di
                            for k in range(KC):
                                nc.tensor.matmul(
                                    ppt[:, di * S : (di + 1) * S],
                                    ftn[:, k * M + i * 128 : k * M + (i + 1) * 128],
                                    qt_sb[:, k * S : (k + 1) * S],
                                    start=(k == 0), stop=(k == KC - 1),
                                )
                        pscr = scr_pool.tile([128, 4 * S], dt.float32, tag="pscr")
                        nc.vector.tensor_tensor(
                            out=pscr[:], in0=ppt[:], in1=oh_w[:, i0 * S : (i0 + 4) * S], op=Alu.mult
                        )
                        nc.vector.tensor_reduce(
                            out=mavg[:, i0 : i0 + 4],
                            in_=pscr.rearrange("p (q s) -> p q s", q=4),
                            axis=mybir.AxisListType.X, op=Alu.add,
                        )

                    # ============ finals (lse ln'd host-side from raw stot2) ============
                    tsum = stat_pool.tile([128, NT], dt.float32, tag="tsum")
                    nc.gpsimd.tensor_tensor(out=tsum[:], in0=diagF[:], in1=mavg[:], op=Alu.add)
                    nc.gpsimd.tensor_tensor(out=acc[:, 0:NT], in0=acc[:, 0:NT], in1=tsum[:], op=Alu.add)
                    nc.gpsimd.tensor_copy(acc[:, 16 + 32 * g : 16 + 32 * (g + 1)], stot2[:])


                stot2 = stat_pool.tile([128, 2 * NT], dt.float32, tag="stot2")
                if g > 0:
                    phase_d(g - 1, *deferred)
                deferred = (fc_bf, ftn, fcn3, bprime, stot2)

                # ============ main: logits GEMM (DR) + exp, n-half outer ============
                for i in range(NT):
                    for h in range(2):
                        lg = lg_psum.tile([128, 1024], dt.float32, tag="lg", name=f"lg{g}_{i}_{h}")
                        for nb in range(2):
                            blk = lg[:, nb * 512 : (nb + 1) * 512]
                            ncol = (2 * h + nb) * 512
                            for kp in (0, 2):
                                nc.tensor.matmul(
                                    blk,
                                    ftn3[:, kp : kp + 2, i * 128 : (i + 1) * 128],
                                    fcn3[:, kp : kp + 2, ncol : ncol + 512],
                                    start=(kp == 0), stop=(kp == 2), perf_mode=DR,
                                )
                        nc.scalar.activation(
                            lg[:], lg[:], Act.Exp, scale=POST,
                            accum_out=stot2[:, 2 * i + h : 2 * i + h + 1],
                        )

            phase_d(GROUPS_PER_CORE - 1, *deferred)
            nc.sync.dma_start(out=out_d[:, :], in_=acc[:])

    nc.compile()
    return nc


def kernel(feat_trainable: np.ndarray, feat_criterion: np.ndarray, grp_masks: np.ndarray) -> np.ndarray:
    from concourse.bass_utils import run_bass_kernel_spmd

    if "nc" not in _CACHE:
        _CACHE["nc"] = _build()
    nc = _CACHE["nc"]

    import ml_dtypes
    ft = np.ascontiguousarray(
        np.asarray(feat_trainable, dtype=np.float32).reshape(B * T, N, C).astype(ml_dtypes.bfloat16))
    fc = np.ascontiguousarray(
        np.asarray(feat_criterion, dtype=np.float32).reshape(B * T, N, C).astype(ml_dtypes.bfloat16))
    gm = np.ascontiguousarray(np.asarray(grp_masks, dtype=np.float32).reshape(B * T, S, N))

    in_maps = []
    for c in range(N_CORES):
        fr = slice(c * FRAMES_PER_CORE, (c + 1) * FRAMES_PER_CORE)
        in_maps.append({
            "ft": np.ascontiguousarray(ft[fr]),
            "fc": np.ascontiguousarray(fc[fr]),
            "gm": np.ascontiguousarray(gm[fr]),
        })

    import time
    last_err = None
    for attempt in range(4):
        try:
            res = run_bass_kernel_spmd(nc, in_maps, list(range(N_CORES)))
            break
        except Exception as e:  # wedged-device recovery: wait and retry
            last_err = e
            time.sleep(20 + 25 * attempt)
    else:
        raise last_err
    total = np.float64(0.0)
    for c in range(N_CORES):
        o = np.asarray(res.results[c]["out"], dtype=np.float64)
        total += POST * o[:, :NT].sum()
        st = o[:, 16:].reshape(128, GROUPS_PER_CORE, NT, 2).sum(axis=-1)
        total -= 2.0 * np.log(st).sum()
    loss = SCALE * total / (G * M) / 2.0
    return np.asarray(loss, dtype=np.float32)


if __name__ == "__main__":
    nc = _build()
    print("build OK")


# revision 9
# speedup vs baseline: 1.9688x; 1.0007x over previous
"""DECConsLoss Trainium2 kernel v3: 8-core data-parallel over groups.

Reference (per group g of G=32, M=2048 tokens, C=512):
  ft_n, fc_n = l2norm(ft), l2norm(fc);  grp[m] = argmax_s masks
  logits = ft_n @ fc_n^T / 0.1;  lse[m] = logsumexp(logits[m,:])
  semi[m] = scale*(mean_{n: grp[n]==grp[m]} logits[m,n] - lse)
  pos[m]  = scale*(logits[m,m] - lse);  loss = mean(semi+pos)/2

v3 pipeline (all primitives hardware-validated):
  - SWDGE casting DMA loads fp32 DRAM -> bf16 SBUF token-major [128,(j,c)]
  - DMA XBAR transposes bf16 SBUF->SBUF -> [128,(j,k,m)] ("alt" layout)
  - ssq via plain bf16 Gram matmuls; diag extracted by tensor_tensor
    against a 4-wide identity + segmented tensor_reduce
  - 16*rsqrt(ssq) via Newton iteration on gpsimd (no ACT table thrash)
  - both tensors scaled to fp8 with broadcast matrices A (j-major out,
    gating per chunk) / B (k-major out, gating per n-half); exp scale is
    then the constant 10/256
  - main loop runs n-half-outer so the first 16 exps only need fc's
    first-half chain; ft's full chain is shorter
  - per-i diag from fp8 DR cross-Grams; masked means via Q/P side-GEMMs
  - exp in-place on PSUM with accum_out row-sums; ln(sums) on the host
Host: loss = SCALE*((10/256)*sum(acc0) - 2*sum(ln(stot))) / (G*M) / 2.
"""

import sys
import numpy as np

for p in ("/opt/trn_rl_repo", "/opt/trn_rl_repo/concourse", "/opt/pypackages"):
    if p not in sys.path:
        sys.path.insert(0, p)

GF = 2
S = 16
N = 1024
C = 512
B, T = 8, 8
G = (B * T) // GF            # 32 groups total
M = GF * N                   # 2048 tokens per group
N_CORES = 8
GROUPS_PER_CORE = G // N_CORES   # 4
FRAMES_PER_CORE = GROUPS_PER_CORE * GF  # 8
TEMP = 0.1
BASE_TEMP = 0.07
INV_TEMP = 1.0 / TEMP
SCALE = -(TEMP / BASE_TEMP)
FS = 16.0                    # fp8 pre-scale on both normalized tensors
POST = INV_TEMP / (FS * FS)  # 10/256: logits = POST * H

NT = M // 128                # 16 token tiles per group
KC = C // 128                # 4 contraction chunks

_CACHE = {}


def _build():
    import math
    import concourse.mybir as mybir
    from concourse import bacc
    from concourse import masks
    from concourse import bass_isa
    from concourse.tile import TileContext

    dt = mybir.dt
    Alu = mybir.AluOpType
    Act = mybir.ActivationFunctionType
    DR = mybir.MatmulPerfMode.DoubleRow

    nc = bacc.Bacc()
    ft_d = nc.declare_dram_parameter("ft", [FRAMES_PER_CORE, N, C], dt.bfloat16, isOutput=False)
    fc_d = nc.declare_dram_parameter("fc", [FRAMES_PER_CORE, N, C], dt.bfloat16, isOutput=False)
    gm_d = nc.declare_dram_parameter("gm", [FRAMES_PER_CORE, S, N], dt.float32, isOutput=False)
    out_d = nc.declare_dram_parameter("out", [128, 16 + 32 * GROUPS_PER_CORE], dt.float32, isOutput=True)

    f8 = dt.float8e4
    bf = dt.bfloat16

    with TileContext(nc) as tc:
        with (
            tc.tile_pool(name="consts", bufs=1) as consts,
            tc.tile_pool(name="tok_pool", bufs=2) as tok_pool,
            tc.tile_pool(name="tT_pool", bufs=4) as tT_pool,
            tc.tile_pool(name="n8_pool", bufs=2) as n8_pool,
            tc.tile_pool(name="ab_pool", bufs=1) as ab_pool,
            tc.tile_pool(name="grp_pool", bufs=2) as grp_pool,
            tc.tile_pool(name="stat_pool", bufs=2) as stat_pool,
            tc.tile_pool(name="scr_pool", bufs=2) as scr_pool,
            tc.tile_pool(name="q_pool", bufs=2) as q_pool,
            tc.tile_pool(name="acc_pool", bufs=1) as acc_pool,
            tc.tile_pool(name="sm_psum", bufs=2, space="PSUM") as sm_psum,
            tc.tile_pool(name="lg_psum", bufs=3, space="PSUM") as lg_psum,
        ):
            # ---- constants ----
            id128_bf = consts.tile([128, 128], bf)
            id128_f32 = consts.tile([128, 128], dt.float32)
            id16_bf = consts.tile([S, S], bf)
            id16_f32 = consts.tile([S, S], dt.float32)
            for t in (id128_bf, id128_f32, id16_bf, id16_f32):
                nc.vector.memset(t[:], 0.0)
                masks.make_identity(nc, t[:], nomemset=True)
            id4_bf = consts.tile([128, 512], bf)
            for q in range(4):
                nc.gpsimd.tensor_copy(id4_bf[:, q * 128 : (q + 1) * 128], id128_bf[:])
            # E8[:, jj*128:(jj+1)*128] = row-jj selector over 8 partitions
            E8 = consts.tile([8, 8 * 128], bf)
            nc.gpsimd.memset(E8[:], 0.0)
            nc.gpsimd.affine_select(
                out=E8[:], in_=E8[:],
                compare_op=mybir.AluOpType.not_equal, fill=1.0, base=0,
                pattern=[[-1, 8], [0, 128]], channel_multiplier=1,
            )

            acc = acc_pool.tile([128, 16 + 32 * GROUPS_PER_CORE], dt.float32)
            nc.vector.memset(acc[:, 0:16], 0.0)

            w0 = FS / math.sqrt(512.0)

            def newton16(ssq, dst, cols):
                """dst[:, cols] = FS*rsqrt(max(ssq,eps)) via 3 Newton steps."""
                sq, dv = ssq[:, cols], dst[:, cols]
                wscr = stat_pool.tile([128, 8], dt.float32, tag="wscr")
                nc.gpsimd.tensor_scalar(
                    out=sq, in0=sq, scalar1=1e-12, scalar2=1.0 / (FS * FS),
                    op0=Alu.max, op1=Alu.mult,
                )
                nc.gpsimd.tensor_scalar(
                    out=dv, in0=sq, scalar1=-0.5 * w0 ** 3, scalar2=1.5 * w0,
                    op0=Alu.mult, op1=Alu.add,
                )
                for _ in range(2):
                    nc.gpsimd.tensor_tensor(out=wscr[:], in0=dv, in1=dv, op=Alu.mult)
                    nc.gpsimd.tensor_tensor(out=wscr[:], in0=wscr[:], in1=sq, op=Alu.mult)
                    nc.gpsimd.tensor_scalar(
                        out=wscr[:], in0=wscr[:], scalar1=-0.5, scalar2=1.5,
                        op0=Alu.mult, op1=Alu.add,
                    )
                    nc.gpsimd.tensor_tensor(out=dv, in0=dv, in1=wscr[:], op=Alu.mult)

            def bcast_half(src, mat, h, g, nm):
                """mat[:, h*1024:(h+1)*1024] <- broadcast rows of src[:, h*8:(h+1)*8]."""
                tps = sm_psum.tile([8, 128], dt.float32, tag="sm", name=f"{nm}tp{g}_{h}")
                nc.tensor.transpose(tps[:], src[:, h * 8 : (h + 1) * 8], id128_f32[:])
                row_sb = q_pool.tile([8, 128], bf, tag=f"{nm}row")
                nc.vector.tensor_copy(row_sb[:], tps[:])
                for half_q in range(2):
                    bps = sm_psum.tile([128, 512], dt.float32, tag="sm", name=f"{nm}ps{g}_{h}_{half_q}")
                    for jj in range(4):
                        j = half_q * 4 + jj
                        nc.tensor.matmul(
                            bps[:, jj * 128 : (jj + 1) * 128],
                            E8[:, j * 128 : (j + 1) * 128], row_sb[:],
                            start=True, stop=True,
                        )
                    nc.vector.tensor_copy(
                        mat[:, h * 1024 + half_q * 512 : h * 1024 + (half_q + 1) * 512], bps[:]
                    )

            def gram_bank(T_bf, ssq, i0, g, nm, src3=None):
                gm = sm_psum.tile([128, 512], dt.float32, tag="sm", name=f"{nm}{g}_{i0}")
                for di in range(4):
                    i = i0 + di
                    if src3 is None:
                        for k in range(KC):
                            sl = slice(k * 1024 + (i % 8) * 128, k * 1024 + (i % 8 + 1) * 128)
                            nc.tensor.matmul(
                                gm[:, di * 128 : (di + 1) * 128],
                                T_bf[i // 8][:, sl], T_bf[i // 8][:, sl],
                                start=(k == 0), stop=(k == KC - 1),
                            )
                    else:
                        lhs3, rhs3 = src3
                        for kp in (0, 2):
                            nc.tensor.matmul(
                                gm[:, di * 128 : (di + 1) * 128],
                                lhs3[:, kp : kp + 2, i * 128 : (i + 1) * 128],
                                rhs3[:, kp : kp + 2, i * 128 : (i + 1) * 128],
                                start=(kp == 0), stop=(kp == 2), perf_mode=DR,
                            )
                scr = scr_pool.tile([128, 512], bf, tag="scr")
                nc.vector.tensor_tensor(out=scr[:], in0=gm[:], in1=id4_bf[:], op=Alu.mult)
                nc.vector.tensor_reduce(
                    out=ssq[:, i0 : i0 + 4],
                    in_=scr.rearrange("p (q m) -> p q m", q=4),
                    axis=mybir.AxisListType.X, op=Alu.add,
                )

            for g in range(GROUPS_PER_CORE):
                # ============ phase B: fc chain ============
                fc_bf = tok_pool.tile([128, NT * C], bf, tag="fcbf", name=f"fcbf{g}")
                fcT = [tT_pool.tile([128, KC * 1024], bf, tag="fcT", name=f"fcT{g}_{hh}")
                       for hh in range(2)]
                ftT = [tT_pool.tile([128, KC * 1024], bf, tag="ftT", name=f"ftT{g}_{hh}")
                       for hh in range(2)]
                ssq_fc = stat_pool.tile([128, NT], dt.float32, tag="ssq_fc")
                ssq_ft = stat_pool.tile([128, NT], dt.float32, tag="ssq_ft")
                bprime = stat_pool.tile([128, NT], dt.float32, tag="bprime")
                B_sb = ab_pool.tile([128, M], bf, tag="Bmat")
                fcn = n8_pool.tile([128, KC * M], f8, tag="fcn", name=f"fcn{g}")
                # per-frame k-major transposes straight from DRAM:
                # T_h[p, k*1024+m] = x[frame h, tok m, c=k*128+p]
                for hh in range(2):
                    nc.sync.dma_start_transpose(
                        out=fcT[hh].rearrange("p (q m) -> p q m", m=1024),
                        in_=fc_d[2 * g + hh],
                    )
                    nc.sync.dma_start_transpose(
                        out=ftT[hh].rearrange("p (q m) -> p q m", m=1024),
                        in_=ft_d[2 * g + hh],
                    )
                nc.sync.dma_start(
                    out=fc_bf.rearrange("p (j c) -> p j c", j=NT),
                    in_=fc_d[2 * g : 2 * g + 2].rearrange("f (jj p) c -> p (f jj) c", p=128),
                )
                # interleave gram banks to match transpose completion order:
                # fcT-h0, ftT-h0, fcT-h1, ftT-h1
                for hh in range(2):
                    for i0 in (hh * 8, hh * 8 + 4):
                        gram_bank(fcT, ssq_fc, i0, g, "cg")
                    for i0 in (hh * 8, hh * 8 + 4):
                        gram_bank(ftT, ssq_ft, i0, g, "fg")
                ahat = stat_pool.tile([128, NT], dt.float32, tag="ahat")
                A_sb = ab_pool.tile([128, M], bf, tag="Amat")
                for h in range(2):
                    newton16(ssq_fc, bprime, slice(h * 8, (h + 1) * 8))
                    bcast_half(bprime, B_sb, h, g, "B")
                    newton16(ssq_ft, ahat, slice(h * 8, (h + 1) * 8))
                    bcast_half(ahat, A_sb, h, g, "A")
                # fcn k-major: out col = k*2048 + n
                for h in range(2):
                    for k in range(KC):
                        eng = nc.vector if k % 2 == 0 else nc.gpsimd
                        eng.tensor_tensor(
                            out=fcn[:, k * M + h * 1024 : k * M + (h + 1) * 1024],
                            in0=fcT[h][:, k * 1024 : (k + 1) * 1024],
                            in1=B_sb[:, h * 1024 : (h + 1) * 1024],
                            op=Alu.mult,
                        )
                fcn3 = fcn.rearrange("p (k n) -> p k n", k=KC)


                # ftn in alt (j-major) layout: col = j*512 + k*128 + m
                ftn = n8_pool.tile([128, KC * M], f8, tag="ftn", name=f"ftn{g}")
                ftn3 = ftn.rearrange("p (k n) -> p k n", k=KC)
                for c in range(4):
                    for k in range(KC):
                        eng = nc.vector if k % 2 == 0 else nc.gpsimd
                        eng.tensor_tensor(
                            out=ftn[:, k * M + c * 512 : k * M + (c + 1) * 512],
                            in0=ftT[c // 2][:, k * 1024 + (c % 2) * 512 : k * 1024 + (c % 2) * 512 + 512],
                            in1=A_sb[:, c * 512 : (c + 1) * 512],
                            op=Alu.mult,
                        )

                def phase_d(g, fc_bf, ftn, fcn3, bprime, stot2):
                    ftn3 = ftn.rearrange("p (k n) -> p k n", k=KC)

                    # grp-mask work is independent; emitted here so its engine use
                    # overlaps the ft/fc chains above.
                    grp_sb = grp_pool.tile([S, M], dt.float32, tag="grp_sb")
                    nc.sync.dma_start(
                        out=grp_sb.rearrange("s (f n) -> s f n", f=GF),
                        in_=gm_d[2 * g : 2 * g + 2].rearrange("f s n -> s f n"),
                    )
                    tpg = sm_psum.tile([128, NT * S], dt.float32, tag="sm", name=f"tpg{g}")
                    for j in range(NT):
                        nc.tensor.transpose(
                            tpg[:, j * S : (j + 1) * S], grp_sb[:, j * 128 : (j + 1) * 128], id16_f32[:]
                        )
                    grpT = grp_pool.tile([128, NT * S], dt.float32, tag="grpT")
                    nc.vector.tensor_copy(grpT[:], tpg[:])
                    rowmax = stat_pool.tile([128, NT], dt.float32, tag="rowmax")
                    nc.vector.tensor_reduce(
                        out=rowmax[:],
                        in_=grpT.rearrange("p (j s) -> p j s", j=NT),
                        axis=mybir.AxisListType.X, op=Alu.max,
                    )
                    oh_f32 = grp_pool.tile([128, NT * S], dt.float32, tag="oh_f32")
                    oh_w = grp_pool.tile([128, NT * S], dt.float32, tag="oh_w")
                    ohsum = stat_pool.tile([128, S], dt.float32, tag="ohsum")
                    cntb = stat_pool.tile([128, S], dt.float32, tag="cntb")
                    for j in range(NT):
                        sl = slice(j * S, (j + 1) * S)
                        nc.gpsimd.tensor_scalar(
                            out=oh_f32[:, sl], in0=grpT[:, sl],
                            scalar1=rowmax[:, j : j + 1], scalar2=None, op0=Alu.is_equal,
                        )
                        if j == 0:
                            nc.gpsimd.tensor_copy(ohsum[:], oh_f32[:, sl])
                        else:
                            nc.gpsimd.tensor_tensor(out=ohsum[:], in0=ohsum[:], in1=oh_f32[:, sl], op=Alu.add)
                    nc.gpsimd.partition_all_reduce(
                        out_ap=cntb[:], in_ap=ohsum[:], channels=128, reduce_op=bass_isa.ReduceOp.add,
                    )
                    nc.gpsimd.tensor_scalar(out=cntb[:], in0=cntb[:], scalar1=1.0, scalar2=None, op0=Alu.max)
                    nc.vector.reciprocal(out=cntb[:], in_=cntb[:])
                    for j in range(NT):
                        sl = slice(j * S, (j + 1) * S)
                        nc.gpsimd.tensor_tensor(out=oh_w[:, sl], in0=oh_f32[:, sl], in1=cntb[:], op=Alu.mult)

                    # cross-Gram diag (ftn alt-layout lhsT, fcn k-major rhs)
                    diagF = stat_pool.tile([128, NT], dt.float32, tag="diagF")
                    for i0 in range(0, NT, 4):
                        gmx = sm_psum.tile([128, 512], dt.float32, tag="sm", name=f"xg{g}_{i0}")
                        for di in range(4):
                            i = i0 + di
                            for kp in (0, 2):
                                nc.tensor.matmul(
                                    gmx[:, di * 128 : (di + 1) * 128],
                                    ftn3[:, kp : kp + 2, i * 128 : (i + 1) * 128],
                                    fcn3[:, kp : kp + 2, i * 128 : (i + 1) * 128],
                                    start=(kp == 0), stop=(kp == 2), perf_mode=DR,
                                )
                        scr = scr_pool.tile([128, 512], bf, tag="scr")
                        nc.vector.tensor_tensor(out=scr[:], in0=gmx[:], in1=id4_bf[:], op=Alu.mult)
                        nc.vector.tensor_reduce(
                            out=diagF[:, i0 : i0 + 4],
                            in_=scr.rearrange("p (q m) -> p q m", q=4),
                            axis=mybir.AxisListType.X, op=Alu.add,
                        )

                    oh_b = grp_pool.tile([128, NT * S], bf, tag="oh_b")
                    for j in range(NT):
                        sl = slice(j * S, (j + 1) * S)
                        nc.gpsimd.tensor_scalar(
                            out=oh_b[:, sl], in0=oh_f32[:, sl],
                            scalar1=bprime[:, j : j + 1], scalar2=None, op0=Alu.mult,
                        )
                    qq = sm_psum.tile([S, C], dt.float32, tag="sm", name=f"qq{g}")
                    for j in range(NT):
                        nc.tensor.matmul(
                            qq[:], oh_b[:, j * S : (j + 1) * S], fc_bf[:, j * C : (j + 1) * C],
                            start=(j == 0), stop=(j == NT - 1),
                        )
                    q_sb = q_pool.tile([S, C], bf, tag="q_sb")
                    nc.vector.tensor_copy(q_sb[:], qq[:])
                    qtp = sm_psum.tile([128, KC * S], bf, tag="sm", name=f"qtp{g}")
                    for k in range(KC):
                        nc.tensor.transpose(
                            qtp[:, k * S : (k + 1) * S], q_sb[:, k * 128 : (k + 1) * 128], id16_bf[:]
                        )
                    qt_sb = q_pool.tile([128, KC * S], bf, tag="qt_sb")
                    nc.vector.tensor_copy(qt_sb[:], qtp[:])

                    mavg = stat_pool.tile([128, NT], dt.float32, tag="mavg")
                    for i0 in range(0, NT, 4):
                        ppt = sm_psum.tile([128, 4 * S], dt.float32, tag="sm", name=f"pp{g}_{i0}")
                        for di in range(4):
                            i = i0 + di
                            for k in range(KC):
                                nc.tensor.matmul(
                                    ppt[:, di * S : (di + 1) * S],
                                    ftn[:, k * M + i * 128 : k * M + (i + 1) * 128],
                                    qt_sb[:, k * S : (k + 1) * S],
                                    start=(k == 0), stop=(k == KC - 1),
                                )
                        pscr = scr_pool.tile([128, 4 * S], dt.float32, tag="pscr")
                        nc.vector.tensor_tensor(
                            out=pscr[:], in0=ppt[:], in1=oh_w[:, i0 * S : (i0 + 4) * S], op=Alu.mult
                        )
                        nc.vector.tensor_reduce(
                            out=mavg[:, i0 : i0 + 4],
                            in_=pscr.rearrange("p (q s) -> p q s", q=4),
                            axis=mybir.AxisListType.X, op=Alu.add,
                        )

                    # ============ finals (lse ln'd host-side from raw stot2) ============
                    tsum = stat_pool.tile([128, NT], dt.float32, tag="tsum")
                    nc.gpsimd.tensor_tensor(out=tsum[:], in0=diagF[:], in1=mavg[:], op=Alu.add)
                    nc.gpsimd.tensor_tensor(out=acc[:, 0:NT], in0=acc[:, 0:NT], in1=tsum[:], op=Alu.add)
                    nc.gpsimd.tensor_copy(acc[:, 16 + 32 * g : 16 + 32 * (g + 1)], stot2[:])


                stot2 = stat_pool.tile([128, 2 * NT], dt.float32, tag="stot2")
                if g > 0:
                    phase_d(g - 1, *deferred)
                deferred = (fc_bf, ftn, fcn3, bprime, stot2)

                # ============ main: logits GEMM (DR) + exp, n-half outer ============
                for i in range(NT):
                    for h in range(2):
                        lg = lg_psum.tile([128, 1024], dt.float32, tag="lg", name=f"lg{g}_{i}_{h}")
                        for nb in range(2):
                            blk = lg[:, nb * 512 : (nb + 1) * 512]
                            ncol = (2 * h + nb) * 512
                            for kp in (0, 2):
                                nc.tensor.matmul(
                                    blk,
                                    ftn3[:, kp : kp + 2, i * 128 : (i + 1) * 128],
                                    fcn3[:, kp : kp + 2, ncol : ncol + 512],
                                    start=(kp == 0), stop=(kp == 2), perf_mode=DR,
                                )
                        nc.scalar.activation(
                            lg[:], lg[:], Act.Exp, scale=POST,
                            accum_out=stot2[:, 2 * i + h : 2 * i + h + 1],
                        )

            phase_d(GROUPS_PER_CORE - 1, *deferred)
            nc.sync.dma_start(out=out_d[:, :], in_=acc[:])

    nc.compile()
    return nc


def kernel(feat_trainable: np.ndarray, feat_criterion: np.ndarray, grp_masks: np.ndarray) -> np.ndarray:
    from concourse.bass_utils import run_bass_kernel_spmd

    if "nc" not in _CACHE:
        _CACHE["nc"] = _build()
    nc = _CACHE["nc"]

    import ml_dtypes
    ft = np.ascontiguousarray(
        np.asarray(feat_trainable, dtype=np.float32).reshape(B * T, N, C).astype(ml_dtypes.bfloat16))
    fc = np.ascontiguousarray(
        np.asarray(feat_criterion, dtype=np.float32).reshape(B * T, N, C).astype(ml_dtypes.bfloat16))
    gm = np.ascontiguousarray(np.asarray(grp_masks, dtype=np.float32).reshape(B * T, S, N))

    in_maps = []
    for c in range(N_CORES):
        fr = slice(c * FRAMES_PER_CORE, (c + 1) * FRAMES_PER_CORE)
        in_maps.append({
            "ft": np.ascontiguousarray(ft[fr]),
            "fc": np.ascontiguousarray(fc[fr]),
            "gm": np.ascontiguousarray(gm[fr]),
        })

    import time
    last_err = None
    for attempt in range(4):
        try:
            res = run_bass_kernel_spmd(nc, in_maps, list(range(N_CORES)))
            break
        except Exception as e:  # wedged-device recovery: wait and retry
            last_err = e
            time.sleep(20 + 25 * attempt)
    else:
        raise last_err
    total = np.float64(0.0)
    for c in range(N_CORES):
        o = np.asarray(res.results[c]["out"], dtype=np.float64)
        total += POST * o[:, :NT].sum()
        st = o[:, 16:].reshape(128, GROUPS_PER_CORE, NT, 2).sum(axis=-1)
        total -= 2.0 * np.log(st).sum()
    loss = SCALE * total / (G * M) / 2.0
    return np.asarray(loss, dtype=np.float32)


if __name__ == "__main__":
    nc = _build()
    print("build OK")
